# revision 8
# baseline (speedup 1.0000x reference)
"""Trainium2 Bass kernel for the DGN message-passing network.

Computation (per batch item b):
    h = relu(x @ enc_w + enc_b)                      [N, H]
    for p in 0..P-1:
        v = relu(h @ wv[p] + bv[p]); q = relu(h @ wq[p] + bq[p]); k = relu(h @ wk[p] + bk[p])
        att = softmax(q @ k.T  masked by mask, axis=-1)
        h = relu((att @ v) @ wo[p] + bo[p])
    y = h @ qw + qb                                  [N, A]

Sharding: data-parallel over the batch dim across 8 NeuronCores (16 items
per core), weights replicated, no cross-core communication.

On-chip layout: everything is kept transposed ([H, N] with H on partitions)
so no transposes are ever required:
  * hT/qT/kT = [H=128, N=512];   projections:  xT = wq.T @ hT  (lhsT = wq)
  * scoresT[m, n] = q[n]·k[m] computed directly as kT-chunk.T @ qT
  * softmax over m (= partition axis of scoresT) is done as
    exp(s)*mask -> rowsum via an all-ones [128,128] matmul (which lands the
    row-sum broadcast across all partitions) -> multiply by reciprocal.
    No max-subtraction: scores of this model are O(8), exp is safe, and
    softmax is shift-invariant so the result matches the reference.
  * v is needed m-on-partitions for the att@v contraction, so it is built
    natively as 4 row chunks packed in one [128, 4*H] PSUM tile; the bias
    (which varies along the free axis there) is preloaded with a single
    K=1 ones x bv4 matmul, then the 4 h-chunk matmuls accumulate on top.

Engine budget per pass-unit (16 items x 2 passes), targeting ~3.7us/unit
on every engine so the PE never starves (HAM stays warm at 2.4 GHz):
  PE : q,k MMs + v preload/4MM + 4 score MMs + 4 rowsum + 4 attv + out MM
  ACT: exp x2 (wide [128,1024]) + ln + exp(-ln) + q-relu (+ enc relu)
  DVE: k-relu, v-relu, h2-relu, otn mult, 2 mask mults (+ y bias-add)
  GPS: 2 mask mults
"""

import numpy as np

import concourse.bass as bass
import concourse.mybir as mybir
import concourse.tile as tile
from concourse.bass import ts
from concourse.bass_utils import run_bass_kernel_spmd

F32 = mybir.dt.float32
BF16 = mybir.dt.bfloat16
AF = mybir.ActivationFunctionType
OP = mybir.AluOpType

N_CORES = 8
B, N, DIN, H, P, A = 128, 512, 64, 128, 2, 16
IPC = B // N_CORES  # batch items per core
NCH = N // 128      # 128-row chunks of the agent dim


def _spill_excess_waits(nc):
    """Walrus codegen has limited sync-wait slots per instruction: a
    self-loading fp32/fp32r Matmult takes only 1 (waits land on its fused
    LDWEIGHTS micro-op) and sequencer ctrl ops (Drain/NoOp) take 4. Spill
    excess waits onto NoOps inserted just before the instruction on the same
    engine - the engine blocks at the NoOp, so ordering semantics are kept.
    """
    counter = [0]

    def make_nop(engine, waits):
        counter[0] += 1
        nop = mybir.InstNoOp(name=f"I-waitspill-{counter[0]}")
        nop.engine = engine
        nop.sync_info = mybir.SyncInfo(on_wait=list(waits), on_update=[])
        return nop

    def sem_clear_insts(inst):
        """This walrus build rejects EVENT_SEMAPHORE_RANGE_CLEAR ("ISA wrong
        length"); expand Tile's tail range-clear into per-sem writes."""
        first = inst.ant_dict["range_first"]
        last = inst.ant_dict["range_last"]
        res = []
        for s in range(first, last + 1):
            counter[0] += 1
            ev = mybir.InstEventSemaphore(name=f"I-semclear-{counter[0]}")
            ev.engine = inst.engine
            ev.sync_info = mybir.SyncInfo(
                on_wait=list(inst.sync_info.on_wait) if (s == first and inst.sync_info) else [],
                on_update=[mybir.SyncUpdate(
                    sync_type="semaphore", id=s,
                    update_mode="sem-wr-imm", update_value=0,
                )],
            )
            res.append(ev)
        return res

    for fn in nc.m.functions:
        for blk in fn.blocks:
            out = []
            for inst in blk.instructions:
                if (type(inst).__name__ == "InstISA"
                        and inst.ant_dict.get("header", {}).get("opcode") == 176):
                    out.extend(sem_clear_insts(inst))
                    continue
                si = inst.sync_info
                waits = list(si.on_wait) if si is not None else []
                limit = 1
                if len(waits) > limit:
                    keep = waits[-limit:] if limit else []
                    spill = waits[: len(waits) - limit]
                    for w in spill:
                        out.append(make_nop(inst.engine, [w]))
                    inst.sync_info.on_wait = keep
                out.append(inst)
            blk.instructions = out


def build_program():
    nc = bass.Bass("TRN2", target_bir_lowering=False, debug=False)

    xt_d = nc.dram_tensor("xt", [IPC, DIN, N], BF16, kind="ExternalInput").ap()
    mt_d = nc.dram_tensor("maskt", [IPC, N, N], BF16, kind="ExternalInput").ap()
    encw_d = nc.dram_tensor("enc_w", [DIN, H], BF16, kind="ExternalInput").ap()
    encb_d = nc.dram_tensor("enc_b", [H, 1], F32, kind="ExternalInput").ap()
    wq_d = nc.dram_tensor("wq", [P, H, H], BF16, kind="ExternalInput").ap()
    wk_d = nc.dram_tensor("wk", [P, H, H], BF16, kind="ExternalInput").ap()
    wv_d = nc.dram_tensor("wv", [P, H, H], BF16, kind="ExternalInput").ap()
    wo_d = nc.dram_tensor("wo", [P, H, H], BF16, kind="ExternalInput").ap()
    bq_d = nc.dram_tensor("bq", [P, H, 1], F32, kind="ExternalInput").ap()
    bk_d = nc.dram_tensor("bk", [P, H, 1], F32, kind="ExternalInput").ap()
    bv4_d = nc.dram_tensor("bv4", [P, 1, NCH * H], BF16, kind="ExternalInput").ap()
    bo_d = nc.dram_tensor("bo", [P, H, 1], F32, kind="ExternalInput").ap()
    qw_d = nc.dram_tensor("qw", [H, A], BF16, kind="ExternalInput").ap()
    ones_d = nc.dram_tensor("ones", [128, 128], BF16, kind="ExternalInput").ap()
    qb_d = nc.dram_tensor("qb", [A, 1], F32, kind="ExternalInput").ap()
    yt_d = nc.dram_tensor("yt", [IPC, A, N], F32, kind="ExternalOutput").ap()

    from contextlib import ExitStack

    with tile.TileContext(nc) as tc:
        with ExitStack() as stack:
            ep = lambda p: stack.enter_context(p)
            wpool = ep(tc.tile_pool(name="weights", bufs=1))
            xpool = ep(tc.tile_pool(name="xin", bufs=3))
            mpool = ep(tc.tile_pool(name="maskin", bufs=3))
            hpool = ep(tc.tile_pool(name="hbuf", bufs=5))
            qpool = ep(tc.tile_pool(name="qbuf", bufs=2))
            kpool = ep(tc.tile_pool(name="kbuf", bufs=2))
            vpool = ep(tc.tile_pool(name="vbuf", bufs=2))
            ppool = ep(tc.tile_pool(name="pbuf", bufs=3))
            rpool = ep(tc.tile_pool(name="rbuf", bufs=2))
            opool = ep(tc.tile_pool(name="obuf", bufs=2))
            ypool = ep(tc.tile_pool(name="ybuf", bufs=2))
            # PSUM: 8 banks exactly. Tags are laid out so a slot's reuse is
            # gated on an EARLY consumer of the previous pass-unit, letting
            # item i+1's matmuls interleave with item i's softmax tail:
            #   qkh  [128,1024] x1 = 2 banks (enc-h, and q|k side by side)
            #   v    [128, 512] x1 = 1 bank
            #   sc   [128,1024] x1 = 2 banks (scores pair tiles)
            #   h2y  [128, 512] x1 = 1 bank (out-proj, and the head)
            #   rs / ot          x1 = 1 bank each
            qkpsum = ep(tc.tile_pool(name="qkpsum", bufs=1, space="PSUM"))
            vpsum = ep(tc.tile_pool(name="vpsum", bufs=1, space="PSUM"))
            spsum = ep(tc.tile_pool(name="spsum", bufs=1, space="PSUM"))
            hpsum = ep(tc.tile_pool(name="hpsum", bufs=1, space="PSUM"))
            rpsum = ep(tc.tile_pool(name="rpsum", bufs=1, space="PSUM"))
            opsum = ep(tc.tile_pool(name="opsum", bufs=1, space="PSUM"))
            # ---- resident weights ----
            encw_t = wpool.tile([DIN, H], BF16, tag="encw")
            nc.sync.dma_start(out=encw_t[:], in_=encw_d[:])
            encb_t = wpool.tile([H, 1], F32, tag="encb")
            nc.sync.dma_start(out=encb_t[:], in_=encb_d[:])
            qw_t = wpool.tile([H, A], BF16, tag="qw")
            nc.sync.dma_start(out=qw_t[:], in_=qw_d[:])
            qb_t = wpool.tile([A, 1], F32, tag="qb")
            nc.sync.dma_start(out=qb_t[:], in_=qb_d[:])
            ones_t = wpool.tile([128, 128], BF16, tag="ones")
            nc.sync.dma_start(out=ones_t[:], in_=ones_d[:])

            wq_t, wk_t, wv_t, wo_t, bq_t, bk_t, bv4_t, bo_t = [], [], [], [], [], [], [], []
            for p in range(P):
                for lst, dram, shape, tag, dt in (
                    (wq_t, wq_d, [H, H], "wq", BF16),
                    (wk_t, wk_d, [H, H], "wk", BF16),
                    (wv_t, wv_d, [H, H], "wv", BF16),
                    (wo_t, wo_d, [H, H], "wo", BF16),
                    (bq_t, bq_d, [H, 1], "bq", F32),
                    (bk_t, bk_d, [H, 1], "bk", F32),
                    (bv4_t, bv4_d, [1, NCH * H], "bv4", BF16),
                    (bo_t, bo_d, [H, 1], "bo", F32),
                ):
                    t = wpool.tile(shape, dt, tag=f"{tag}{p}")
                    nc.sync.dma_start(out=t[:], in_=dram[p])
                    lst.append(t)

            # ---- per-item pipeline ----
            for i in range(IPC):
                xt_t = xpool.tile([DIN, N], BF16, tag="xt")
                nc.sync.dma_start(out=xt_t[:], in_=xt_d[i])
                mt_t = mpool.tile([128, NCH * N], BF16, tag="mt")
                nc.sync.dma_start(
                    out=mt_t[:], in_=mt_d[i].rearrange("(c p) n -> p c n", c=NCH)
                )

                # encoder: hT = relu(enc_w.T @ xT + enc_b)
                hp = qkpsum.tile([H, N], F32, tag="qkh")
                nc.tensor.matmul(hp[:], lhsT=(encw_t[:]), rhs=(xt_t[:]), start=True, stop=True)
                hT = hpool.tile([H, N], BF16, tag="h")
                nc.scalar.activation(hT[:], hp[:], AF.Relu, bias=encb_t[:])

                for p in range(P):
                    # projections q and k side by side in one 2-bank tile;
                    # q-relu on ACT, k-relu on DVE
                    qkp = qkpsum.tile([H, 2 * N], F32, tag="qkh")
                    nc.tensor.matmul(qkp[:, 0:N], lhsT=(wq_t[p][:]), rhs=(hT[:]), start=True, stop=True)
                    nc.tensor.matmul(qkp[:, N:], lhsT=(wk_t[p][:]), rhs=(hT[:]), start=True, stop=True)
                    qT = qpool.tile([H, N], BF16, tag="q")
                    nc.scalar.activation(qT[:], qkp[:, 0:N], AF.Relu, bias=bq_t[p][:])
                    kT = kpool.tile([H, N], BF16, tag="k")
                    nc.vector.tensor_scalar(
                        out=kT[:], in0=qkp[:, N:], scalar1=bk_t[p][:], scalar2=0.0,
                        op0=OP.add, op1=OP.max,
                    )

                    # v in natural [m, h] layout: all 4 row chunks in one
                    # [128, 4H] PSUM tile. Bias varies along the FREE axis
                    # here, so preload ones(x)bv4 via a K=1 matmul, then
                    # accumulate the 4 hT-chunk @ wv products on top.
                    vp = vpsum.tile([128, NCH * H], F32, tag="v")
                    nc.tensor.matmul(
                        vp[:], lhsT=(ones_t[0:1, :]), rhs=(bv4_t[p][:]),
                        start=True, stop=False,
                    )
                    for c in range(NCH):
                        nc.tensor.matmul(
                            vp[:, ts(c, H)], lhsT=(hT[:, ts(c, 128)]), rhs=(wv_t[p][:]),
                            start=False, stop=(c == NCH - 1),
                        )
                    vn = vpool.tile([128, NCH * H], BF16, tag="v")
                    nc.vector.tensor_scalar_max(vn[:], vp[:], 0.0)

                    # scoresT[m, n] in two wide [128, 2N] PSUM tiles; exp is
                    # one wide ACTIVATE per pair -> pT (bf16, SBUF)
                    pT = ppool.tile([128, NCH * N], BF16, tag="p")
                    for j in range(NCH // 2):
                        scp = spsum.tile([128, 2 * N], F32, tag="sc")
                        for cc in range(2):
                            c = 2 * j + cc
                            nc.tensor.matmul(
                                scp[:, ts(cc, N)], lhsT=(kT[:, ts(c, 128)]), rhs=(qT[:]),
                                start=True, stop=True,
                            )
                        nc.scalar.activation(pT[:, ts(j, 2 * N)], scp[:], AF.Exp)
                    # mask multiply (elementwise): 2 chunks DVE, 2 GpSimd
                    for c in range(NCH):
                        eng = nc.vector if c % 2 == 0 else nc.gpsimd
                        eng.tensor_tensor(
                            out=pT[:, ts(c, N)], in0=pT[:, ts(c, N)],
                            in1=mt_t[:, ts(c, N)], op=OP.mult,
                        )

                    # rowsum over m via all-ones matmul (broadcast to all parts)
                    rs = rpsum.tile([128, N], F32, tag="rs")
                    for c in range(NCH):
                        nc.tensor.matmul(
                            rs[:], lhsT=(ones_t[:]), rhs=(pT[:, ts(c, N)]),
                            start=(c == 0), stop=(c == NCH - 1),
                        )
                    # outT = v.T @ p  (accumulate over m chunks)
                    ot = opsum.tile([H, N], F32, tag="ot")
                    for c in range(NCH):
                        nc.tensor.matmul(
                            ot[:], lhsT=(vn[:, ts(c, H)]), rhs=(pT[:, ts(c, N)]),
                            start=(c == 0), stop=(c == NCH - 1),
                        )
                    # 1/rowsum = exp(-ln(rowsum)): Ln and Exp live in the
                    # same ACT table set as the softmax Exp, so no table
                    # switching (Reciprocal would thrash 2.7us loads).
                    lnr = rpool.tile([H, N], F32, tag="lnr")
                    nc.scalar.activation(lnr[:], rs[:], AF.Ln)
                    recipb = rpool.tile([H, N], F32, tag="recip")
                    nc.scalar.activation(recipb[:], lnr[:], AF.Exp, scale=-1.0)
                    otn = opool.tile([H, N], BF16, tag="otn")
                    nc.vector.tensor_tensor(out=otn[:], in0=ot[:], in1=recipb[:], op=OP.mult)

                    # out projection -> new hT
                    h2p = hpsum.tile([H, N], F32, tag="h2y")
                    nc.tensor.matmul(h2p[:], lhsT=(wo_t[p][:]), rhs=(otn[:]), start=True, stop=True)
                    hT = hpool.tile([H, N], BF16, tag="h")
                    nc.vector.tensor_scalar(
                        out=hT[:], in0=h2p[:], scalar1=bo_t[p][:], scalar2=0.0,
                        op0=OP.add, op1=OP.max,
                    )

                # Q head: yT = qw.T @ hT + qb
                yp = hpsum.tile([A, N], F32, tag="h2y")
                nc.tensor.matmul(yp[:], lhsT=(qw_t[:]), rhs=(hT[:]), start=True, stop=True)
                yt_t = ypool.tile([A, N], F32, tag="y")
                nc.vector.tensor_scalar_add(yt_t[:], yp[:], qb_t[:])
                nc.sync.dma_start(out=yt_d[i], in_=yt_t[:])

    _spill_excess_waits(nc)
    return nc


_prog_cache = None


def _get_program():
    global _prog_cache
    if _prog_cache is None:
        _prog_cache = build_program()
    return _prog_cache


def _make_in_maps(x, mask, enc_w, enc_b, wv, bv, wk, bk, wq, bq, wo, bo, qw, qb):
    import ml_dtypes
    bf = lambda a: np.ascontiguousarray(np.asarray(a, dtype=np.float32).astype(ml_dtypes.bfloat16))
    f = lambda a: np.ascontiguousarray(np.asarray(a, dtype=np.float32))
    x, mask = f(x), f(mask)
    shared = {
        "enc_w": bf(enc_w),
        "enc_b": f(enc_b).reshape(H, 1),
        "wq": bf(wq),
        "wk": bf(wk),
        "wv": bf(wv),
        "wo": bf(wo),
        "bq": f(bq).reshape(P, H, 1),
        "bk": f(bk).reshape(P, H, 1),
        "bv4": np.ascontiguousarray(np.tile(bf(bv), (1, NCH)).reshape(P, 1, NCH * H)),
        "bo": f(bo).reshape(P, H, 1),
        "qw": bf(qw),
        "ones": np.ones((128, 128), dtype=ml_dtypes.bfloat16),
        "qb": f(qb).reshape(A, 1),
    }
    in_maps = []
    for c in range(N_CORES):
        sl = slice(c * IPC, (c + 1) * IPC)
        in_maps.append({
            "xt": np.ascontiguousarray(x[sl].transpose(0, 2, 1).astype(ml_dtypes.bfloat16)),
            "maskt": np.ascontiguousarray(mask[sl].transpose(0, 2, 1).astype(ml_dtypes.bfloat16)),
            **shared,
        })
    return in_maps


def run(trace=False, **inputs):
    nc = _get_program()
    in_maps = _make_in_maps(**inputs)
    res = run_bass_kernel_spmd(nc, in_maps, list(range(N_CORES)), trace=trace)
    y = np.concatenate(
        [r["yt"].transpose(0, 2, 1) for r in res.results], axis=0
    ).astype(np.float32)
    return y, res


def kernel(**inputs):
    y, _ = run(trace=False, **inputs)
    return y


# revision 13
# speedup vs baseline: 1.0689x; 1.0689x over previous
"""Trainium2 Bass kernel for the DGN message-passing network.

Computation (per batch item b):
    h = relu(x @ enc_w + enc_b)                      [N, H]
    for p in 0..P-1:
        v = relu(h @ wv[p] + bv[p]); q = relu(h @ wq[p] + bq[p]); k = relu(h @ wk[p] + bk[p])
        att = softmax(q @ k.T  masked by mask, axis=-1)
        h = relu((att @ v) @ wo[p] + bo[p])
    y = h @ qw + qb                                  [N, A]

Sharding: data-parallel over the batch dim across 8 NeuronCores (16 items
per core), weights replicated, no cross-core communication.

On-chip layout: everything is kept transposed ([H, N] with H on partitions)
so no transposes are ever required:
  * hT/qT/kT = [H=128, N=512];   projections:  xT = wq.T @ hT  (lhsT = wq)
  * scoresT[m, n] = q[n]·k[m] computed directly as kT-chunk.T @ qT
  * softmax over m (= partition axis of scoresT) is done as
    exp(s)*mask -> rowsum via an all-ones [128,128] matmul (which lands the
    row-sum broadcast across all partitions) -> multiply by reciprocal.
    No max-subtraction: scores of this model are O(8), exp is safe, and
    softmax is shift-invariant so the result matches the reference.
  * v is needed m-on-partitions for the att@v contraction, so it is built
    natively as 4 row chunks packed in one [128, 4*H] PSUM tile; the bias
    (which varies along the free axis there) is preloaded with a single
    K=1 ones x bv4 matmul, then the 4 h-chunk matmuls accumulate on top.

Engine budget per pass-unit (16 items x 2 passes), targeting ~3.7us/unit
on every engine so the PE never starves (HAM stays warm at 2.4 GHz):
  PE : q,k MMs + v preload/4MM + 4 score MMs + 4 rowsum + 4 attv + out MM
  ACT: exp x2 (wide [128,1024]) + ln + exp(-ln) + q-relu (+ enc relu)
  DVE: k-relu, v-relu, h2-relu, otn mult, 2 mask mults (+ y bias-add)
  GPS: 2 mask mults
"""

import numpy as np

import concourse.bass as bass
import concourse.mybir as mybir
import concourse.tile as tile
from concourse.bass import ts
from concourse.bass_utils import run_bass_kernel_spmd

F32 = mybir.dt.float32
BF16 = mybir.dt.bfloat16
AF = mybir.ActivationFunctionType
OP = mybir.AluOpType

N_CORES = 8
B, N, DIN, H, P, A = 128, 512, 64, 128, 2, 16
IPC = B // N_CORES  # batch items per core
NCH = N // 128      # 128-row chunks of the agent dim


def _spill_excess_waits(nc):
    """Walrus codegen has limited sync-wait slots per instruction: a
    self-loading fp32/fp32r Matmult takes only 1 (waits land on its fused
    LDWEIGHTS micro-op) and sequencer ctrl ops (Drain/NoOp) take 4. Spill
    excess waits onto NoOps inserted just before the instruction on the same
    engine - the engine blocks at the NoOp, so ordering semantics are kept.
    """
    counter = [0]

    def make_nop(engine, waits):
        counter[0] += 1
        nop = mybir.InstNoOp(name=f"I-waitspill-{counter[0]}")
        nop.engine = engine
        nop.sync_info = mybir.SyncInfo(on_wait=list(waits), on_update=[])
        return nop

    def sem_clear_insts(inst):
        """This walrus build rejects EVENT_SEMAPHORE_RANGE_CLEAR ("ISA wrong
        length"); expand Tile's tail range-clear into per-sem writes."""
        first = inst.ant_dict["range_first"]
        last = inst.ant_dict["range_last"]
        res = []
        for s in range(first, last + 1):
            counter[0] += 1
            ev = mybir.InstEventSemaphore(name=f"I-semclear-{counter[0]}")
            ev.engine = inst.engine
            ev.sync_info = mybir.SyncInfo(
                on_wait=list(inst.sync_info.on_wait) if (s == first and inst.sync_info) else [],
                on_update=[mybir.SyncUpdate(
                    sync_type="semaphore", id=s,
                    update_mode="sem-wr-imm", update_value=0,
                )],
            )
            res.append(ev)
        return res

    for fn in nc.m.functions:
        for blk in fn.blocks:
            out = []
            for inst in blk.instructions:
                if (type(inst).__name__ == "InstISA"
                        and inst.ant_dict.get("header", {}).get("opcode") == 176):
                    out.extend(sem_clear_insts(inst))
                    continue
                si = inst.sync_info
                waits = list(si.on_wait) if si is not None else []
                limit = 1
                if len(waits) > limit:
                    keep = waits[-limit:] if limit else []
                    spill = waits[: len(waits) - limit]
                    for w in spill:
                        out.append(make_nop(inst.engine, [w]))
                    inst.sync_info.on_wait = keep
                out.append(inst)
            blk.instructions = out


def build_program():
    nc = bass.Bass("TRN2", target_bir_lowering=False, debug=False)

    xt_d = nc.dram_tensor("xt", [IPC, DIN, N], BF16, kind="ExternalInput").ap()
    mt_d = nc.dram_tensor("maskt", [IPC, N, N], BF16, kind="ExternalInput").ap()
    encw_d = nc.dram_tensor("enc_w", [DIN, H], BF16, kind="ExternalInput").ap()
    encb_d = nc.dram_tensor("enc_b", [H, 1], F32, kind="ExternalInput").ap()
    wq_d = nc.dram_tensor("wq", [P, H, H], BF16, kind="ExternalInput").ap()
    wk_d = nc.dram_tensor("wk", [P, H, H], BF16, kind="ExternalInput").ap()
    wv_d = nc.dram_tensor("wv", [P, H, H], BF16, kind="ExternalInput").ap()
    wo_d = nc.dram_tensor("wo", [P, H, H], BF16, kind="ExternalInput").ap()
    bq_d = nc.dram_tensor("bq", [P, H, 1], F32, kind="ExternalInput").ap()
    bk_d = nc.dram_tensor("bk", [P, H, 1], F32, kind="ExternalInput").ap()
    bv4_d = nc.dram_tensor("bv4", [P, 1, NCH * H], BF16, kind="ExternalInput").ap()
    bo_d = nc.dram_tensor("bo", [P, H, 1], F32, kind="ExternalInput").ap()
    qw_d = nc.dram_tensor("qw", [H, A], BF16, kind="ExternalInput").ap()
    ones_d = nc.dram_tensor("ones", [128, 128], BF16, kind="ExternalInput").ap()
    qb_d = nc.dram_tensor("qb", [A, 1], F32, kind="ExternalInput").ap()
    yt_d = nc.dram_tensor("yt", [IPC, A, N], F32, kind="ExternalOutput").ap()

    from contextlib import ExitStack

    with tile.TileContext(nc) as tc:
        with ExitStack() as stack:
            ep = lambda p: stack.enter_context(p)
            wpool = ep(tc.tile_pool(name="weights", bufs=1))
            xpool = ep(tc.tile_pool(name="xin", bufs=2))
            mpool = ep(tc.tile_pool(name="maskin", bufs=4))
            hpool = ep(tc.tile_pool(name="hbuf", bufs=2))
            qpool = ep(tc.tile_pool(name="qbuf", bufs=3))
            kpool = ep(tc.tile_pool(name="kbuf", bufs=3))
            vpool = ep(tc.tile_pool(name="vbuf", bufs=3))
            ppool = ep(tc.tile_pool(name="pbuf", bufs=3))
            rpool = ep(tc.tile_pool(name="rbuf", bufs=2))
            opool = ep(tc.tile_pool(name="obuf", bufs=2))
            ypool = ep(tc.tile_pool(name="ybuf", bufs=2))
            # PSUM: 8 banks exactly, four 2-bank [128,1024] tags x 1 buf.
            # Slot reuse is gated on an early consumer of the previous
            # lockstep stage, so the pair pipeline never blocks on banks:
            #   qkh: enc2(g) -> qk(i0) -> qk(i1) -> h2p2 -> ... -> enc2(g+1)
            #   sc : v(i0) -> v(i1) -> sc(i0,j0..j1) -> sc(i1,j0..j1) -> yp2
            #   rs2/ot2: pair-wide rowsum / att@v accumulators
            qkpsum = ep(tc.tile_pool(name="qkpsum", bufs=1, space="PSUM"))
            spsum = ep(tc.tile_pool(name="spsum", bufs=1, space="PSUM"))
            rpsum = ep(tc.tile_pool(name="rpsum", bufs=1, space="PSUM"))
            opsum = ep(tc.tile_pool(name="opsum", bufs=1, space="PSUM"))
            # ---- resident weights ----
            encw_t = wpool.tile([DIN, H], BF16, tag="encw")
            nc.sync.dma_start(out=encw_t[:], in_=encw_d[:])
            encb_t = wpool.tile([H, 1], F32, tag="encb")
            nc.sync.dma_start(out=encb_t[:], in_=encb_d[:])
            qw_t = wpool.tile([H, A], BF16, tag="qw")
            nc.sync.dma_start(out=qw_t[:], in_=qw_d[:])
            qb_t = wpool.tile([A, 1], F32, tag="qb")
            nc.sync.dma_start(out=qb_t[:], in_=qb_d[:])
            ones_t = wpool.tile([128, 128], BF16, tag="ones")
            nc.sync.dma_start(out=ones_t[:], in_=ones_d[:])

            wq_t, wk_t, wv_t, wo_t, bq_t, bk_t, bv4_t, bo_t = [], [], [], [], [], [], [], []
            for p in range(P):
                for lst, dram, shape, tag, dt in (
                    (wq_t, wq_d, [H, H], "wq", BF16),
                    (wk_t, wk_d, [H, H], "wk", BF16),
                    (wv_t, wv_d, [H, H], "wv", BF16),
                    (wo_t, wo_d, [H, H], "wo", BF16),
                    (bq_t, bq_d, [H, 1], "bq", F32),
                    (bk_t, bk_d, [H, 1], "bk", F32),
                    (bv4_t, bv4_d, [1, NCH * H], "bv4", BF16),
                    (bo_t, bo_d, [H, 1], "bo", F32),
                ):
                    t = wpool.tile(shape, dt, tag=f"{tag}{p}")
                    nc.sync.dma_start(out=t[:], in_=dram[p])
                    lst.append(t)

            # ---- lockstep item-PAIR pipeline ----
            # Two items march through the stages together so that when one
            # item's softmax chain (exp -> mask -> norm) occupies ACT/DVE,
            # the PE runs the sibling item's matmuls. The linear layers
            # whose weights are shared (encoder, out-proj, head) batch the
            # pair into one [*, 2N] matmul (bf16 moving limit is 1024).
            for g in range(IPC // 2):
                xp_t = xpool.tile([DIN, 2 * N], BF16, tag="xt")
                nc.sync.dma_start(
                    out=xp_t[:],
                    in_=xt_d[2 * g : 2 * g + 2].rearrange("i d n -> d i n"),
                )
                mt_t = []
                for it in range(2):
                    t = mpool.tile([128, NCH * N], BF16, tag="mt")
                    nc.sync.dma_start(
                        out=t[:],
                        in_=mt_d[2 * g + it].rearrange("(c p) n -> p c n", c=NCH),
                    )
                    mt_t.append(t)

                # encoder for the pair: hpair = relu(enc_w.T @ [x0|x1] + b)
                ep = qkpsum.tile([H, 2 * N], F32, tag="qkh")
                for it in range(2):
                    nc.tensor.matmul(
                        ep[:, ts(it, N)], lhsT=(encw_t[:]), rhs=(xp_t[:, ts(it, N)]),
                        start=True, stop=True,
                    )
                hpair = hpool.tile([H, 2 * N], BF16, tag="h")
                nc.vector.tensor_scalar(
                    out=hpair[:], in0=ep[:], scalar1=encb_t[:], scalar2=0.0,
                    op0=OP.add, op1=OP.max,
                )

                for p in range(P):
                    # ---- A: q/k/v projections, lockstep over the pair ----
                    qT, kT, vn = [], [], []
                    for it in range(2):
                        hT = hpair[:, ts(it, N)]
                        qkp = qkpsum.tile([H, 2 * N], F32, tag="qkh")
                        nc.tensor.matmul(qkp[:, 0:N], lhsT=(wq_t[p][:]), rhs=(hT), start=True, stop=True)
                        nc.tensor.matmul(qkp[:, N:], lhsT=(wk_t[p][:]), rhs=(hT), start=True, stop=True)
                        qt = qpool.tile([H, N], BF16, tag="q")
                        nc.scalar.activation(qt[:], qkp[:, 0:N], AF.Relu, bias=bq_t[p][:])
                        qT.append(qt)
                        kt = kpool.tile([H, N], BF16, tag="k")
                        nc.vector.tensor_scalar(
                            out=kt[:], in0=qkp[:, N:], scalar1=bk_t[p][:], scalar2=0.0,
                            op0=OP.add, op1=OP.max,
                        )
                        kT.append(kt)

                        # v in natural [m, h] layout: 4 row chunks in one
                        # [128, 4H] PSUM tile; bias varies along the FREE
                        # axis, so preload ones(x)bv4 via a K=1 matmul.
                        vp = spsum.tile([128, NCH * H], F32, tag="sc")
                        nc.tensor.matmul(
                            vp[:], lhsT=(ones_t[0:1, :]), rhs=(bv4_t[p][:]),
                            start=True, stop=False,
                        )
                        for c in range(NCH):
                            nc.tensor.matmul(
                                vp[:, ts(c, H)], lhsT=(hT[:, ts(c, 128)]), rhs=(wv_t[p][:]),
                                start=False, stop=(c == NCH - 1),
                            )
                        vt = vpool.tile([128, NCH * H], BF16, tag="v")
                        nc.vector.tensor_scalar_max(vt[:], vp[:], 0.0)
                        vn.append(vt)

                    # ---- B: scores + exp + mask, lockstep ----
                    pT = []
                    for it in range(2):
                        pt = ppool.tile([128, NCH * N], BF16, tag="p")
                        for j in range(NCH // 2):
                            scp = spsum.tile([128, 2 * N], F32, tag="sc")
                            for cc in range(2):
                                c = 2 * j + cc
                                nc.tensor.matmul(
                                    scp[:, ts(cc, N)], lhsT=(kT[it][:, ts(c, 128)]),
                                    rhs=(qT[it][:]), start=True, stop=True,
                                )
                            nc.scalar.activation(pt[:, ts(j, 2 * N)], scp[:], AF.Exp)
                        # mask multiply: chunks 0-1 on DVE, 2-3 on GpSimd
                        for c in range(NCH):
                            eng = nc.vector if c < 2 else nc.gpsimd
                            eng.tensor_tensor(
                                out=pt[:, ts(c, N)], in0=pt[:, ts(c, N)],
                                in1=mt_t[it][:, ts(c, N)], op=OP.mult,
                            )
                        pT.append(pt)

                    # ---- C: rowsum + att@v, pair-wide PSUM tiles ----
                    rs2 = rpsum.tile([128, 2 * N], F32, tag="rs2")
                    for it in range(2):
                        for c in range(NCH):
                            nc.tensor.matmul(
                                rs2[:, it * N : (it + 1) * N], lhsT=(ones_t[:]),
                                rhs=(pT[it][:, ts(c, N)]),
                                start=(c == 0), stop=(c == NCH - 1),
                            )
                    ot2 = opsum.tile([H, 2 * N], F32, tag="ot2")
                    for it in range(2):
                        for c in range(NCH):
                            nc.tensor.matmul(
                                ot2[:, it * N : (it + 1) * N], lhsT=(vn[it][:, ts(c, H)]),
                                rhs=(pT[it][:, ts(c, N)]),
                                start=(c == 0), stop=(c == NCH - 1),
                            )

                    # ---- D: normalize + out-projection, pair-batched ----
                    # 1/rowsum = exp(-ln(rowsum)): Ln/Exp share the loaded
                    # ACT table set (Reciprocal would thrash 2.7us loads).
                    lnr2 = rpool.tile([H, 2 * N], F32, tag="lnr")
                    nc.scalar.activation(lnr2[:], rs2[:], AF.Ln)
                    recip2 = rpool.tile([H, 2 * N], F32, tag="recip")
                    nc.scalar.activation(recip2[:], lnr2[:], AF.Exp, scale=-1.0)
                    otn2 = opool.tile([H, 2 * N], BF16, tag="otn")
                    nc.vector.tensor_tensor(out=otn2[:], in0=ot2[:], in1=recip2[:], op=OP.mult)

                    h2p2 = qkpsum.tile([H, 2 * N], F32, tag="qkh")
                    for it in range(2):
                        nc.tensor.matmul(
                            h2p2[:, ts(it, N)], lhsT=(wo_t[p][:]), rhs=(otn2[:, ts(it, N)]),
                            start=True, stop=True,
                        )
                    # overwrite hpair in place (all pass-p readers are done)
                    nc.vector.tensor_scalar(
                        out=hpair[:], in0=h2p2[:], scalar1=bo_t[p][:], scalar2=0.0,
                        op0=OP.add, op1=OP.max,
                    )

                # Q head for the pair: y2 = qw.T @ hpair + qb
                yp2 = spsum.tile([A, 2 * N], F32, tag="sc")
                for it in range(2):
                    nc.tensor.matmul(
                        yp2[:, ts(it, N)], lhsT=(qw_t[:]), rhs=(hpair[:, ts(it, N)]),
                        start=True, stop=True,
                    )
                y2 = ypool.tile([A, 2 * N], F32, tag="y")
                nc.vector.tensor_scalar_add(y2[:], yp2[:], qb_t[:])
                nc.sync.dma_start(
                    out=yt_d[2 * g : 2 * g + 2].rearrange("i a n -> a i n"),
                    in_=y2[:],
                )

    _spill_excess_waits(nc)
    return nc


_prog_cache = None


def _get_program():
    global _prog_cache
    if _prog_cache is None:
        _prog_cache = build_program()
    return _prog_cache


def _make_in_maps(x, mask, enc_w, enc_b, wv, bv, wk, bk, wq, bq, wo, bo, qw, qb):
    import ml_dtypes
    bf = lambda a: np.ascontiguousarray(np.asarray(a, dtype=np.float32).astype(ml_dtypes.bfloat16))
    f = lambda a: np.ascontiguousarray(np.asarray(a, dtype=np.float32))
    x, mask = f(x), f(mask)
    shared = {
        "enc_w": bf(enc_w),
        "enc_b": f(enc_b).reshape(H, 1),
        "wq": bf(wq),
        "wk": bf(wk),
        "wv": bf(wv),
        "wo": bf(wo),
        "bq": f(bq).reshape(P, H, 1),
        "bk": f(bk).reshape(P, H, 1),
        "bv4": np.ascontiguousarray(np.tile(bf(bv), (1, NCH)).reshape(P, 1, NCH * H)),
        "bo": f(bo).reshape(P, H, 1),
        "qw": bf(qw),
        "ones": np.ones((128, 128), dtype=ml_dtypes.bfloat16),
        "qb": f(qb).reshape(A, 1),
    }
    in_maps = []
    for c in range(N_CORES):
        sl = slice(c * IPC, (c + 1) * IPC)
        in_maps.append({
            "xt": np.ascontiguousarray(x[sl].transpose(0, 2, 1).astype(ml_dtypes.bfloat16)),
            "maskt": np.ascontiguousarray(mask[sl].transpose(0, 2, 1).astype(ml_dtypes.bfloat16)),
            **shared,
        })
    return in_maps


def run(trace=False, **inputs):
    nc = _get_program()
    in_maps = _make_in_maps(**inputs)
    res = run_bass_kernel_spmd(nc, in_maps, list(range(N_CORES)), trace=trace)
    y = np.concatenate(
        [r["yt"].transpose(0, 2, 1) for r in res.results], axis=0
    ).astype(np.float32)
    return y, res


def kernel(**inputs):
    y, _ = run(trace=False, **inputs)
    return y


# revision 16
# speedup vs baseline: 1.1053x; 1.0341x over previous
"""Trainium2 Bass kernel for the DGN message-passing network.

Computation (per batch item b):
    h = relu(x @ enc_w + enc_b)                      [N, H]
    for p in 0..P-1:
        v = relu(h @ wv[p] + bv[p]); q = relu(h @ wq[p] + bq[p]); k = relu(h @ wk[p] + bk[p])
        att = softmax(q @ k.T  masked by mask, axis=-1)
        h = relu((att @ v) @ wo[p] + bo[p])
    y = h @ qw + qb                                  [N, A]

Sharding: data-parallel over the batch dim across 8 NeuronCores (16 items
per core), weights replicated, no cross-core communication.

On-chip layout: everything is kept transposed ([H, N] with H on partitions)
so no transposes are ever required:
  * hT/qT/kT = [H=128, N=512];   projections:  xT = wq.T @ hT  (lhsT = wq)
  * scoresT[m, n] = q[n]·k[m] computed directly as kT-chunk.T @ qT
  * softmax over m (= partition axis of scoresT) is done as
    exp(s)*mask -> rowsum via an all-ones [128,128] matmul (which lands the
    row-sum broadcast across all partitions) -> multiply by reciprocal.
    No max-subtraction: scores of this model are O(8), exp is safe, and
    softmax is shift-invariant so the result matches the reference.
  * v is needed m-on-partitions for the att@v contraction, so it is built
    natively as 4 row chunks packed in one [128, 4*H] PSUM tile; the bias
    (which varies along the free axis there) is preloaded with a single
    K=1 ones x bv4 matmul, then the 4 h-chunk matmuls accumulate on top.

Engine budget per pass-unit (16 items x 2 passes), targeting ~3.7us/unit
on every engine so the PE never starves (HAM stays warm at 2.4 GHz):
  PE : q,k MMs + v preload/4MM + 4 score MMs + 4 rowsum + 4 attv + out MM
  ACT: exp x2 (wide [128,1024]) + ln + exp(-ln) + q-relu (+ enc relu)
  DVE: k-relu, v-relu, h2-relu, otn mult, 2 mask mults (+ y bias-add)
  GPS: 2 mask mults
"""

import numpy as np

import concourse.bass as bass
import concourse.mybir as mybir
import concourse.tile as tile
from concourse.bass import ts
from concourse.bass_utils import run_bass_kernel_spmd

F32 = mybir.dt.float32
BF16 = mybir.dt.bfloat16
AF = mybir.ActivationFunctionType
OP = mybir.AluOpType

N_CORES = 8
B, N, DIN, H, P, A = 128, 512, 64, 128, 2, 16
IPC = B // N_CORES  # batch items per core
NCH = N // 128      # 128-row chunks of the agent dim


def _spill_excess_waits(nc):
    """Walrus codegen has limited sync-wait slots per instruction: a
    self-loading fp32/fp32r Matmult takes only 1 (waits land on its fused
    LDWEIGHTS micro-op) and sequencer ctrl ops (Drain/NoOp) take 4. Spill
    excess waits onto NoOps inserted just before the instruction on the same
    engine - the engine blocks at the NoOp, so ordering semantics are kept.
    """
    counter = [0]

    def make_nop(engine, waits):
        counter[0] += 1
        nop = mybir.InstNoOp(name=f"I-waitspill-{counter[0]}")
        nop.engine = engine
        nop.sync_info = mybir.SyncInfo(on_wait=list(waits), on_update=[])
        return nop

    def sem_clear_insts(inst):
        """This walrus build rejects EVENT_SEMAPHORE_RANGE_CLEAR ("ISA wrong
        length"); expand Tile's tail range-clear into per-sem writes."""
        first = inst.ant_dict["range_first"]
        last = inst.ant_dict["range_last"]
        res = []
        for s in range(first, last + 1):
            counter[0] += 1
            ev = mybir.InstEventSemaphore(name=f"I-semclear-{counter[0]}")
            ev.engine = inst.engine
            ev.sync_info = mybir.SyncInfo(
                on_wait=list(inst.sync_info.on_wait) if (s == first and inst.sync_info) else [],
                on_update=[mybir.SyncUpdate(
                    sync_type="semaphore", id=s,
                    update_mode="sem-wr-imm", update_value=0,
                )],
            )
            res.append(ev)
        return res

    for fn in nc.m.functions:
        for blk in fn.blocks:
            out = []
            for inst in blk.instructions:
                if (type(inst).__name__ == "InstISA"
                        and inst.ant_dict.get("header", {}).get("opcode") == 176):
                    out.extend(sem_clear_insts(inst))
                    continue
                si = inst.sync_info
                waits = list(si.on_wait) if si is not None else []
                limit = 1
                if len(waits) > limit:
                    keep = waits[-limit:] if limit else []
                    spill = waits[: len(waits) - limit]
                    for w in spill:
                        out.append(make_nop(inst.engine, [w]))
                    inst.sync_info.on_wait = keep
                out.append(inst)
            blk.instructions = out


def build_program():
    nc = bass.Bass("TRN2", target_bir_lowering=False, debug=False)

    xt_d = nc.dram_tensor("xt", [IPC, DIN, N], BF16, kind="ExternalInput").ap()
    mt_d = nc.dram_tensor("maskt", [IPC, N, N], BF16, kind="ExternalInput").ap()
    encw_d = nc.dram_tensor("enc_w", [DIN, H], BF16, kind="ExternalInput").ap()
    encb_d = nc.dram_tensor("enc_b", [H, 1], F32, kind="ExternalInput").ap()
    wq_d = nc.dram_tensor("wq", [P, H, H], BF16, kind="ExternalInput").ap()
    wk_d = nc.dram_tensor("wk", [P, H, H], BF16, kind="ExternalInput").ap()
    wv_d = nc.dram_tensor("wv", [P, H, H], BF16, kind="ExternalInput").ap()
    wo_d = nc.dram_tensor("wo", [P, H, H], BF16, kind="ExternalInput").ap()
    bq_d = nc.dram_tensor("bq", [P, H, 1], F32, kind="ExternalInput").ap()
    bk_d = nc.dram_tensor("bk", [P, H, 1], F32, kind="ExternalInput").ap()
    bv4_d = nc.dram_tensor("bv4", [P, 1, NCH * H], BF16, kind="ExternalInput").ap()
    bo_d = nc.dram_tensor("bo", [P, H, 1], F32, kind="ExternalInput").ap()
    qw_d = nc.dram_tensor("qw", [H, A], BF16, kind="ExternalInput").ap()
    ones_d = nc.dram_tensor("ones", [128, 128], BF16, kind="ExternalInput").ap()
    qb_d = nc.dram_tensor("qb", [A, 1], F32, kind="ExternalInput").ap()
    yt_d = nc.dram_tensor("yt", [IPC, A, N], F32, kind="ExternalOutput").ap()

    from contextlib import ExitStack

    with tile.TileContext(nc) as tc:
        with ExitStack() as stack:
            ep = lambda p: stack.enter_context(p)
            wpool = ep(tc.tile_pool(name="weights", bufs=1))
            xpool = ep(tc.tile_pool(name="xin", bufs=2))
            mpool = ep(tc.tile_pool(name="maskin", bufs=4))
            hpool = ep(tc.tile_pool(name="hbuf", bufs=2))
            qpool = ep(tc.tile_pool(name="qbuf", bufs=3))
            kpool = ep(tc.tile_pool(name="kbuf", bufs=3))
            vpool = ep(tc.tile_pool(name="vbuf", bufs=3))
            ppool = ep(tc.tile_pool(name="pbuf", bufs=3))
            rpool = ep(tc.tile_pool(name="rbuf", bufs=2))
            opool = ep(tc.tile_pool(name="obuf", bufs=3))
            ypool = ep(tc.tile_pool(name="ybuf", bufs=2))
            # PSUM: 8 banks exactly, four 2-bank [128,1024] tags x 1 buf.
            # Slot reuse is gated on an early consumer of the previous
            # lockstep stage, so the pair pipeline never blocks on banks:
            #   qkh: enc2(g) -> qk(i0) -> qk(i1) -> h2p2 -> ... -> enc2(g+1)
            #   sc : v(i0) -> v(i1) -> sc(i0,j0..j1) -> sc(i1,j0..j1) -> yp2
            #   rs2/ot2: pair-wide rowsum / att@v accumulators
            qkpsum = ep(tc.tile_pool(name="qkpsum", bufs=1, space="PSUM"))
            spsum = ep(tc.tile_pool(name="spsum", bufs=1, space="PSUM"))
            rpsum = ep(tc.tile_pool(name="rpsum", bufs=1, space="PSUM"))
            opsum = ep(tc.tile_pool(name="opsum", bufs=1, space="PSUM"))
            # ---- resident weights ----
            encw_t = wpool.tile([DIN, H], BF16, tag="encw")
            nc.sync.dma_start(out=encw_t[:], in_=encw_d[:])
            encb_t = wpool.tile([H, 1], F32, tag="encb")
            nc.sync.dma_start(out=encb_t[:], in_=encb_d[:])
            qw_t = wpool.tile([H, A], BF16, tag="qw")
            nc.sync.dma_start(out=qw_t[:], in_=qw_d[:])
            qb_t = wpool.tile([A, 1], F32, tag="qb")
            nc.sync.dma_start(out=qb_t[:], in_=qb_d[:])
            ones_t = wpool.tile([128, 128], BF16, tag="ones")
            nc.sync.dma_start(out=ones_t[:], in_=ones_d[:])

            wq_t, wk_t, wv_t, wo_t, bq_t, bk_t, bv4_t, bo_t = [], [], [], [], [], [], [], []
            for p in range(P):
                for lst, dram, shape, tag, dt in (
                    (wq_t, wq_d, [H, H], "wq", BF16),
                    (wk_t, wk_d, [H, H], "wk", BF16),
                    (wv_t, wv_d, [H, H], "wv", BF16),
                    (wo_t, wo_d, [H, H], "wo", BF16),
                    (bq_t, bq_d, [H, 1], "bq", F32),
                    (bk_t, bk_d, [H, 1], "bk", F32),
                    (bv4_t, bv4_d, [1, NCH * H], "bv4", BF16),
                    (bo_t, bo_d, [H, 1], "bo", F32),
                ):
                    t = wpool.tile(shape, dt, tag=f"{tag}{p}")
                    nc.sync.dma_start(out=t[:], in_=dram[p])
                    lst.append(t)

            # ---- lockstep item-PAIR pipeline ----
            # Two items march through the stages together so that when one
            # item's softmax chain (exp -> mask -> norm) occupies ACT/DVE,
            # the PE runs the sibling item's matmuls. The linear layers
            # whose weights are shared (encoder, out-proj, head) batch the
            # pair into one [*, 2N] matmul (bf16 moving limit is 1024).
            for g in range(IPC // 2):
                xp_t = xpool.tile([DIN, 2 * N], BF16, tag="xt")
                nc.sync.dma_start(
                    out=xp_t[:],
                    in_=xt_d[2 * g : 2 * g + 2].rearrange("i d n -> d i n"),
                )
                mt_t = []
                for it in range(2):
                    t = mpool.tile([128, NCH * N], BF16, tag="mt")
                    nc.sync.dma_start(
                        out=t[:],
                        in_=mt_d[2 * g + it].rearrange("(c p) n -> p c n", c=NCH),
                    )
                    mt_t.append(t)

                # encoder for the pair: hpair = relu(enc_w.T @ [x0|x1] + b)
                ep = qkpsum.tile([H, 2 * N], F32, tag="qkh")
                for it in range(2):
                    nc.tensor.matmul(
                        ep[:, ts(it, N)], lhsT=(encw_t[:]), rhs=(xp_t[:, ts(it, N)]),
                        start=True, stop=True,
                    )
                hpair = hpool.tile([H, 2 * N], BF16, tag="h")
                nc.vector.tensor_scalar(
                    out=hpair[:], in0=ep[:], scalar1=encb_t[:], scalar2=0.0,
                    op0=OP.add, op1=OP.max,
                )

                for p in range(P):
                    # ---- A: q/k/v projections, lockstep over the pair ----
                    qT, kT, vn = [], [], []
                    for it in range(2):
                        hT = hpair[:, ts(it, N)]
                        qkp = qkpsum.tile([H, 2 * N], F32, tag="qkh")
                        nc.tensor.matmul(qkp[:, 0:N], lhsT=(wq_t[p][:]), rhs=(hT), start=True, stop=True)
                        nc.tensor.matmul(qkp[:, N:], lhsT=(wk_t[p][:]), rhs=(hT), start=True, stop=True)
                        qt = qpool.tile([H, N], BF16, tag="q")
                        nc.scalar.activation(qt[:], qkp[:, 0:N], AF.Relu, bias=bq_t[p][:])
                        qT.append(qt)
                        kt = kpool.tile([H, N], BF16, tag="k")
                        nc.vector.tensor_scalar(
                            out=kt[:], in0=qkp[:, N:], scalar1=bk_t[p][:], scalar2=0.0,
                            op0=OP.add, op1=OP.max,
                        )
                        kT.append(kt)

                        # v in natural [m, h] layout: 4 row chunks in one
                        # [128, 4H] PSUM tile; bias varies along the FREE
                        # axis, so preload ones(x)bv4 via a K=1 matmul.
                        vp = spsum.tile([128, NCH * H], F32, tag="sc")
                        nc.tensor.matmul(
                            vp[:], lhsT=(ones_t[0:1, :]), rhs=(bv4_t[p][:]),
                            start=True, stop=False,
                        )
                        for c in range(NCH):
                            nc.tensor.matmul(
                                vp[:, ts(c, H)], lhsT=(hT[:, ts(c, 128)]), rhs=(wv_t[p][:]),
                                start=False, stop=(c == NCH - 1),
                            )
                        vt = vpool.tile([128, NCH * H], BF16, tag="v")
                        nc.vector.tensor_scalar_max(vt[:], vp[:], 0.0)
                        vn.append(vt)

                    # ---- B: scores + exp + mask, lockstep ----
                    pT = []
                    for it in range(2):
                        pt = ppool.tile([128, NCH * N], BF16, tag="p")
                        for j in range(NCH // 2):
                            scp = spsum.tile([128, 2 * N], F32, tag="sc")
                            for cc in range(2):
                                c = 2 * j + cc
                                nc.tensor.matmul(
                                    scp[:, ts(cc, N)], lhsT=(kT[it][:, ts(c, 128)]),
                                    rhs=(qT[it][:]), start=True, stop=True,
                                )
                            nc.scalar.activation(pt[:, ts(j, 2 * N)], scp[:], AF.Exp)
                        # mask multiply: GpSimd is ~2.7x slower per chunk, so
                        # give it the EARLY chunks (ready right after exp j0)
                        # and let the fast DVE take the late ones - the last
                        # chunk then lands as soon as exp j1 retires.
                        for c in range(NCH):
                            eng = nc.gpsimd if c < 2 else nc.vector
                            eng.tensor_tensor(
                                out=pt[:, ts(c, N)], in0=pt[:, ts(c, N)],
                                in1=mt_t[it][:, ts(c, N)], op=OP.mult,
                            )
                        pT.append(pt)

                    # ---- C: rowsum + att@v, pair-wide PSUM tiles ----
                    rs2 = rpsum.tile([128, 2 * N], F32, tag="rs2")
                    for it in range(2):
                        for c in range(NCH):
                            nc.tensor.matmul(
                                rs2[:, it * N : (it + 1) * N], lhsT=(ones_t[:]),
                                rhs=(pT[it][:, ts(c, N)]),
                                start=(c == 0), stop=(c == NCH - 1),
                            )
                    ot2 = opsum.tile([H, 2 * N], F32, tag="ot2")
                    for it in range(2):
                        for c in range(NCH):
                            nc.tensor.matmul(
                                ot2[:, it * N : (it + 1) * N], lhsT=(vn[it][:, ts(c, H)]),
                                rhs=(pT[it][:, ts(c, N)]),
                                start=(c == 0), stop=(c == NCH - 1),
                            )

                    # ---- D: normalize + out-projection ----
                    # 1/rowsum = exp(-ln(rowsum)): Ln/Exp share the loaded
                    # ACT table set (Reciprocal would thrash 2.7us loads).
                    # ln/recip are pair-wide (one ACT op each); the rest is
                    # per-item so item i0's h2-relu lands first and unblocks
                    # the next pass's A-stage while i1 is still normalizing.
                    lnr2 = rpool.tile([H, 2 * N], F32, tag="lnr")
                    nc.scalar.activation(lnr2[:], rs2[:], AF.Ln)
                    recip2 = rpool.tile([H, 2 * N], F32, tag="recip")
                    nc.scalar.activation(recip2[:], lnr2[:], AF.Exp, scale=-1.0)
                    for it in range(2):
                        otn = opool.tile([H, N], BF16, tag="otn")
                        nc.vector.tensor_tensor(
                            out=otn[:], in0=ot2[:, ts(it, N)],
                            in1=recip2[:, ts(it, N)], op=OP.mult,
                        )
                        h2p = qkpsum.tile([H, N], F32, tag="qkh")
                        nc.tensor.matmul(h2p[:], lhsT=(wo_t[p][:]), rhs=(otn[:]), start=True, stop=True)
                        # overwrite hpair half in place (pass-p readers done)
                        nc.vector.tensor_scalar(
                            out=hpair[:, ts(it, N)], in0=h2p[:], scalar1=bo_t[p][:],
                            scalar2=0.0, op0=OP.add, op1=OP.max,
                        )

                # Q head for the pair: y2 = qw.T @ hpair + qb
                yp2 = spsum.tile([A, 2 * N], F32, tag="sc")
                for it in range(2):
                    nc.tensor.matmul(
                        yp2[:, ts(it, N)], lhsT=(qw_t[:]), rhs=(hpair[:, ts(it, N)]),
                        start=True, stop=True,
                    )
                y2 = ypool.tile([A, 2 * N], F32, tag="y")
                nc.vector.tensor_scalar_add(y2[:], yp2[:], qb_t[:])
                nc.sync.dma_start(
                    out=yt_d[2 * g : 2 * g + 2].rearrange("i a n -> a i n"),
                    in_=y2[:],
                )

    _spill_excess_waits(nc)
    return nc


_prog_cache = None


def _get_program():
    global _prog_cache
    if _prog_cache is None:
        _prog_cache = build_program()
    return _prog_cache


def _make_in_maps(x, mask, enc_w, enc_b, wv, bv, wk, bk, wq, bq, wo, bo, qw, qb):
    import ml_dtypes
    bf = lambda a: np.ascontiguousarray(np.asarray(a, dtype=np.float32).astype(ml_dtypes.bfloat16))
    f = lambda a: np.ascontiguousarray(np.asarray(a, dtype=np.float32))
    x, mask = f(x), f(mask)
    shared = {
        "enc_w": bf(enc_w),
        "enc_b": f(enc_b).reshape(H, 1),
        "wq": bf(wq),
        "wk": bf(wk),
        "wv": bf(wv),
        "wo": bf(wo),
        "bq": f(bq).reshape(P, H, 1),
        "bk": f(bk).reshape(P, H, 1),
        "bv4": np.ascontiguousarray(np.tile(bf(bv), (1, NCH)).reshape(P, 1, NCH * H)),
        "bo": f(bo).reshape(P, H, 1),
        "qw": bf(qw),
        "ones": np.ones((128, 128), dtype=ml_dtypes.bfloat16),
        "qb": f(qb).reshape(A, 1),
    }
    in_maps = []
    for c in range(N_CORES):
        sl = slice(c * IPC, (c + 1) * IPC)
        in_maps.append({
            "xt": np.ascontiguousarray(x[sl].transpose(0, 2, 1).astype(ml_dtypes.bfloat16)),
            "maskt": np.ascontiguousarray(mask[sl].transpose(0, 2, 1).astype(ml_dtypes.bfloat16)),
            **shared,
        })
    return in_maps


def run(trace=False, **inputs):
    nc = _get_program()
    in_maps = _make_in_maps(**inputs)
    res = run_bass_kernel_spmd(nc, in_maps, list(range(N_CORES)), trace=trace)
    y = np.concatenate(
        [r["yt"].transpose(0, 2, 1) for r in res.results], axis=0
    ).astype(np.float32)
    return y, res


def kernel(**inputs):
    y, _ = run(trace=False, **inputs)
    return y


# revision 18
# speedup vs baseline: 1.3414x; 1.2137x over previous
"""Trainium2 Bass kernel for the DGN message-passing network.

Computation (per batch item b):
    h = relu(x @ enc_w + enc_b)                      [N, H]
    for p in 0..P-1:
        v = relu(h @ wv[p] + bv[p]); q = relu(h @ wq[p] + bq[p]); k = relu(h @ wk[p] + bk[p])
        att = softmax(q @ k.T  masked by mask, axis=-1)
        h = relu((att @ v) @ wo[p] + bo[p])
    y = h @ qw + qb                                  [N, A]

Sharding: data-parallel over the batch dim across 8 NeuronCores (16 items
per core), weights replicated, no cross-core communication.

On-chip layout: everything is kept transposed ([H, N] with H on partitions)
so no transposes are ever required:
  * hT/qT/kT = [H=128, N=512];   projections:  xT = wq.T @ hT  (lhsT = wq)
  * scoresT[m, n] = q[n]·k[m] computed directly as kT-chunk.T @ qT
  * softmax over m (= partition axis of scoresT) is done as
    exp(s)*mask -> rowsum via an all-ones [128,128] matmul (which lands the
    row-sum broadcast across all partitions) -> multiply by reciprocal.
    No max-subtraction: scores of this model are O(8), exp is safe, and
    softmax is shift-invariant so the result matches the reference.
  * v is needed m-on-partitions for the att@v contraction, so it is built
    natively as 4 row chunks packed in one [128, 4*H] PSUM tile; the bias
    (which varies along the free axis there) is preloaded with a single
    K=1 ones x bv4 matmul, then the 4 h-chunk matmuls accumulate on top.

Engine budget per pass-unit (16 items x 2 passes), targeting ~3.7us/unit
on every engine so the PE never starves (HAM stays warm at 2.4 GHz):
  PE : q,k MMs + v preload/4MM + 4 score MMs + 4 rowsum + 4 attv + out MM
  ACT: exp x2 (wide [128,1024]) + ln + exp(-ln) + q-relu (+ enc relu)
  DVE: k-relu, v-relu, h2-relu, otn mult, 2 mask mults (+ y bias-add)
  GPS: 2 mask mults
"""

import numpy as np

import concourse.bass as bass
import concourse.mybir as mybir
import concourse.tile as tile
from concourse.bass import ts
from concourse.bass_utils import run_bass_kernel_spmd

F32 = mybir.dt.float32
BF16 = mybir.dt.bfloat16
AF = mybir.ActivationFunctionType
OP = mybir.AluOpType

N_CORES = 8
B, N, DIN, H, P, A = 128, 512, 64, 128, 2, 16
IPC = B // N_CORES  # batch items per core
NCH = N // 128      # 128-row chunks of the agent dim


def _spill_excess_waits(nc):
    """Walrus codegen has limited sync-wait slots per instruction: a
    self-loading fp32/fp32r Matmult takes only 1 (waits land on its fused
    LDWEIGHTS micro-op) and sequencer ctrl ops (Drain/NoOp) take 4. Spill
    excess waits onto NoOps inserted just before the instruction on the same
    engine - the engine blocks at the NoOp, so ordering semantics are kept.
    """
    counter = [0]

    def make_nop(engine, waits):
        counter[0] += 1
        nop = mybir.InstNoOp(name=f"I-waitspill-{counter[0]}")
        nop.engine = engine
        nop.sync_info = mybir.SyncInfo(on_wait=list(waits), on_update=[])
        return nop

    def sem_clear_insts(inst):
        """This walrus build rejects EVENT_SEMAPHORE_RANGE_CLEAR ("ISA wrong
        length"); expand Tile's tail range-clear into per-sem writes."""
        first = inst.ant_dict["range_first"]
        last = inst.ant_dict["range_last"]
        res = []
        for s in range(first, last + 1):
            counter[0] += 1
            ev = mybir.InstEventSemaphore(name=f"I-semclear-{counter[0]}")
            ev.engine = inst.engine
            ev.sync_info = mybir.SyncInfo(
                on_wait=list(inst.sync_info.on_wait) if (s == first and inst.sync_info) else [],
                on_update=[mybir.SyncUpdate(
                    sync_type="semaphore", id=s,
                    update_mode="sem-wr-imm", update_value=0,
                )],
            )
            res.append(ev)
        return res

    for fn in nc.m.functions:
        for blk in fn.blocks:
            out = []
            for inst in blk.instructions:
                if (type(inst).__name__ == "InstISA"
                        and inst.ant_dict.get("header", {}).get("opcode") == 176):
                    out.extend(sem_clear_insts(inst))
                    continue
                si = inst.sync_info
                waits = list(si.on_wait) if si is not None else []
                limit = 1
                if len(waits) > limit:
                    keep = waits[-limit:] if limit else []
                    spill = waits[: len(waits) - limit]
                    for w in spill:
                        out.append(make_nop(inst.engine, [w]))
                    inst.sync_info.on_wait = keep
                out.append(inst)
            blk.instructions = out


def build_program():
    nc = bass.Bass("TRN2", target_bir_lowering=False, debug=False)

    xt_d = nc.dram_tensor("xt", [IPC, DIN, N], BF16, kind="ExternalInput").ap()
    mt_d = nc.dram_tensor("maskt", [IPC, N, N], BF16, kind="ExternalInput").ap()
    encw_d = nc.dram_tensor("enc_w", [DIN, H], BF16, kind="ExternalInput").ap()
    encb_d = nc.dram_tensor("enc_b", [H, 1], F32, kind="ExternalInput").ap()
    wq_d = nc.dram_tensor("wq", [P, H, H], BF16, kind="ExternalInput").ap()
    wk_d = nc.dram_tensor("wk", [P, H, H], BF16, kind="ExternalInput").ap()
    wv_d = nc.dram_tensor("wv", [P, H, H], BF16, kind="ExternalInput").ap()
    wo_d = nc.dram_tensor("wo", [P, H, H], BF16, kind="ExternalInput").ap()
    bq_d = nc.dram_tensor("bq", [P, H, 1], F32, kind="ExternalInput").ap()
    bk_d = nc.dram_tensor("bk", [P, H, 1], F32, kind="ExternalInput").ap()
    bv4_d = nc.dram_tensor("bv4", [P, 1, NCH * H], BF16, kind="ExternalInput").ap()
    bo_d = nc.dram_tensor("bo", [P, H, 1], F32, kind="ExternalInput").ap()
    qw_d = nc.dram_tensor("qw", [H, A], BF16, kind="ExternalInput").ap()
    ones_d = nc.dram_tensor("ones", [128, 128], BF16, kind="ExternalInput").ap()
    qb_d = nc.dram_tensor("qb", [A, 1], F32, kind="ExternalInput").ap()
    yt_d = nc.dram_tensor("yt", [IPC, A, N], F32, kind="ExternalOutput").ap()

    from contextlib import ExitStack

    with tile.TileContext(nc) as tc:
        with ExitStack() as stack:
            ep = lambda p: stack.enter_context(p)
            wpool = ep(tc.tile_pool(name="weights", bufs=1))
            xpool = ep(tc.tile_pool(name="xin", bufs=2))
            mpool = ep(tc.tile_pool(name="maskin", bufs=4))
            hpool = ep(tc.tile_pool(name="hbuf", bufs=2))
            qpool = ep(tc.tile_pool(name="qbuf", bufs=3))
            kpool = ep(tc.tile_pool(name="kbuf", bufs=3))
            vpool = ep(tc.tile_pool(name="vbuf", bufs=3))
            ppool = ep(tc.tile_pool(name="pbuf", bufs=3))
            rpool = ep(tc.tile_pool(name="rbuf", bufs=2))
            opool = ep(tc.tile_pool(name="obuf", bufs=3))
            ypool = ep(tc.tile_pool(name="ybuf", bufs=2))
            # PSUM: 8 banks exactly, four 2-bank [128,1024] tags x 1 buf.
            # Slot reuse is gated on an early consumer of the previous
            # lockstep stage, so the pair pipeline never blocks on banks:
            #   qkh: enc2(g) -> qk(i0) -> qk(i1) -> h2p2 -> ... -> enc2(g+1)
            #   sc : v(i0) -> v(i1) -> sc(i0,j0..j1) -> sc(i1,j0..j1) -> yp2
            #   rs2/ot2: pair-wide rowsum / att@v accumulators
            qkpsum = ep(tc.tile_pool(name="qkpsum", bufs=1, space="PSUM"))
            spsum = ep(tc.tile_pool(name="spsum", bufs=1, space="PSUM"))
            rpsum = ep(tc.tile_pool(name="rpsum", bufs=1, space="PSUM"))
            opsum = ep(tc.tile_pool(name="opsum", bufs=1, space="PSUM"))
            # ---- resident weights ----
            encw_t = wpool.tile([DIN, H], BF16, tag="encw")
            nc.sync.dma_start(out=encw_t[:], in_=encw_d[:])
            encb_t = wpool.tile([H, 1], F32, tag="encb")
            nc.sync.dma_start(out=encb_t[:], in_=encb_d[:])
            qw_t = wpool.tile([H, A], BF16, tag="qw")
            nc.sync.dma_start(out=qw_t[:], in_=qw_d[:])
            qb_t = wpool.tile([A, 1], F32, tag="qb")
            nc.sync.dma_start(out=qb_t[:], in_=qb_d[:])
            ones_t = wpool.tile([128, 128], BF16, tag="ones")
            nc.sync.dma_start(out=ones_t[:], in_=ones_d[:])

            wq_t, wk_t, wv_t, wo_t, bq_t, bk_t, bv4_t, bo_t = [], [], [], [], [], [], [], []
            for p in range(P):
                for lst, dram, shape, tag, dt in (
                    (wq_t, wq_d, [H, H], "wq", BF16),
                    (wk_t, wk_d, [H, H], "wk", BF16),
                    (wv_t, wv_d, [H, H], "wv", BF16),
                    (wo_t, wo_d, [H, H], "wo", BF16),
                    (bq_t, bq_d, [H, 1], "bq", F32),
                    (bk_t, bk_d, [H, 1], "bk", F32),
                    (bv4_t, bv4_d, [1, NCH * H], "bv4", BF16),
                    (bo_t, bo_d, [H, 1], "bo", F32),
                ):
                    t = wpool.tile(shape, dt, tag=f"{tag}{p}")
                    nc.sync.dma_start(out=t[:], in_=dram[p])
                    lst.append(t)

            # ---- two-stream skewed item-PAIR pipeline ----
            # Items march in lockstep pairs, and TWO pairs are kept in
            # flight, skewed by SKEW=5 stage-slots (pair g at pass-1 while
            # pair g+1 runs pass-0). Each pair's serial normalize chain
            # (ln -> exp -> otn -> h2) then overlaps the other pair's
            # matmul-dense stages, so the PE never starves and the HAM
            # clock gate stays warm. PSUM tags rotate just-in-time under
            # this emission order (verified per-tag; all bufs=1).
            st = [dict() for _ in range(IPC // 2)]

            def stage_entry(g):
                s = st[g]
                xp_t = xpool.tile([DIN, 2 * N], BF16, tag="xt")
                nc.sync.dma_start(
                    out=xp_t[:],
                    in_=xt_d[2 * g : 2 * g + 2].rearrange("i d n -> d i n"),
                )
                s["mt"] = []
                for it in range(2):
                    t = mpool.tile([128, NCH * N], BF16, tag="mt")
                    nc.sync.dma_start(
                        out=t[:],
                        in_=mt_d[2 * g + it].rearrange("(c p) n -> p c n", c=NCH),
                    )
                    s["mt"].append(t)
                # encoder for the pair: hpair = relu(enc_w.T @ [x0|x1] + b)
                ep = qkpsum.tile([H, 2 * N], F32, tag="qkh")
                for it in range(2):
                    nc.tensor.matmul(
                        ep[:, ts(it, N)], lhsT=(encw_t[:]), rhs=(xp_t[:, ts(it, N)]),
                        start=True, stop=True,
                    )
                hpair = hpool.tile([H, 2 * N], BF16, tag="h")
                nc.vector.tensor_scalar(
                    out=hpair[:], in0=ep[:], scalar1=encb_t[:], scalar2=0.0,
                    op0=OP.add, op1=OP.max,
                )
                s["h"] = hpair

            def stage_A(g, p):
                s = st[g]
                hpair = s["h"]
                s["q"], s["k"], s["v"] = [], [], []
                for it in range(2):
                    hT = hpair[:, ts(it, N)]
                    qkp = qkpsum.tile([H, 2 * N], F32, tag="qkh")
                    nc.tensor.matmul(qkp[:, 0:N], lhsT=(wq_t[p][:]), rhs=(hT), start=True, stop=True)
                    nc.tensor.matmul(qkp[:, N:], lhsT=(wk_t[p][:]), rhs=(hT), start=True, stop=True)
                    qt = qpool.tile([H, N], BF16, tag="q")
                    nc.scalar.activation(qt[:], qkp[:, 0:N], AF.Relu, bias=bq_t[p][:])
                    s["q"].append(qt)
                    kt = kpool.tile([H, N], BF16, tag="k")
                    nc.vector.tensor_scalar(
                        out=kt[:], in0=qkp[:, N:], scalar1=bk_t[p][:], scalar2=0.0,
                        op0=OP.add, op1=OP.max,
                    )
                    s["k"].append(kt)
                    # v in natural [m, h] layout: 4 row chunks in one
                    # [128, 4H] PSUM tile; bias varies along the FREE
                    # axis, so preload ones(x)bv4 via a K=1 matmul.
                    vp = spsum.tile([128, NCH * H], F32, tag="sc")
                    nc.tensor.matmul(
                        vp[:], lhsT=(ones_t[0:1, :]), rhs=(bv4_t[p][:]),
                        start=True, stop=False,
                    )
                    for c in range(NCH):
                        nc.tensor.matmul(
                            vp[:, ts(c, H)], lhsT=(hT[:, ts(c, 128)]), rhs=(wv_t[p][:]),
                            start=False, stop=(c == NCH - 1),
                        )
                    vt = vpool.tile([128, NCH * H], BF16, tag="v")
                    nc.vector.tensor_scalar_max(vt[:], vp[:], 0.0)
                    s["v"].append(vt)

            def stage_B(g, p):
                s = st[g]
                s["p"] = []
                for it in range(2):
                    pt = ppool.tile([128, NCH * N], BF16, tag="p")
                    for j in range(NCH // 2):
                        scp = spsum.tile([128, 2 * N], F32, tag="sc")
                        for cc in range(2):
                            c = 2 * j + cc
                            nc.tensor.matmul(
                                scp[:, ts(cc, N)], lhsT=(s["k"][it][:, ts(c, 128)]),
                                rhs=(s["q"][it][:]), start=True, stop=True,
                            )
                        nc.scalar.activation(pt[:, ts(j, 2 * N)], scp[:], AF.Exp)
                    # mask multiply: GpSimd is ~2.7x slower per chunk, so
                    # give it the EARLY chunks (ready right after exp j0)
                    # and let the fast DVE take the late ones - the last
                    # chunk then lands as soon as exp j1 retires.
                    for c in range(NCH):
                        eng = nc.gpsimd if c < 2 else nc.vector
                        eng.tensor_tensor(
                            out=pt[:, ts(c, N)], in0=pt[:, ts(c, N)],
                            in1=s["mt"][it][:, ts(c, N)], op=OP.mult,
                        )
                    s["p"].append(pt)

            def stage_C(g, p):
                s = st[g]
                rs2 = rpsum.tile([128, 2 * N], F32, tag="rs2")
                for it in range(2):
                    for c in range(NCH):
                        nc.tensor.matmul(
                            rs2[:, it * N : (it + 1) * N], lhsT=(ones_t[:]),
                            rhs=(s["p"][it][:, ts(c, N)]),
                            start=(c == 0), stop=(c == NCH - 1),
                        )
                ot2 = opsum.tile([H, 2 * N], F32, tag="ot2")
                for it in range(2):
                    for c in range(NCH):
                        nc.tensor.matmul(
                            ot2[:, it * N : (it + 1) * N], lhsT=(s["v"][it][:, ts(c, H)]),
                            rhs=(s["p"][it][:, ts(c, N)]),
                            start=(c == 0), stop=(c == NCH - 1),
                        )
                s["rs2"], s["ot2"] = rs2, ot2

            def stage_D(g, p):
                s = st[g]
                # 1/rowsum = exp(-ln(rowsum)): Ln/Exp share the loaded ACT
                # table set (Reciprocal would thrash 2.7us table loads).
                # ln/recip are pair-wide (one ACT op each); the rest is
                # per-item so item i0's h2-relu lands first.
                lnr2 = rpool.tile([H, 2 * N], F32, tag="lnr")
                nc.scalar.activation(lnr2[:], s["rs2"][:], AF.Ln)
                recip2 = rpool.tile([H, 2 * N], F32, tag="recip")
                nc.scalar.activation(recip2[:], lnr2[:], AF.Exp, scale=-1.0)
                for it in range(2):
                    otn = opool.tile([H, N], BF16, tag="otn")
                    nc.vector.tensor_tensor(
                        out=otn[:], in0=s["ot2"][:, ts(it, N)],
                        in1=recip2[:, ts(it, N)], op=OP.mult,
                    )
                    h2p = qkpsum.tile([H, N], F32, tag="qkh")
                    nc.tensor.matmul(h2p[:], lhsT=(wo_t[p][:]), rhs=(otn[:]), start=True, stop=True)
                    # overwrite hpair half in place (pass-p readers done)
                    nc.vector.tensor_scalar(
                        out=s["h"][:, ts(it, N)], in0=h2p[:], scalar1=bo_t[p][:],
                        scalar2=0.0, op0=OP.add, op1=OP.max,
                    )

            def stage_head(g):
                s = st[g]
                yp2 = spsum.tile([A, 2 * N], F32, tag="sc")
                for it in range(2):
                    nc.tensor.matmul(
                        yp2[:, ts(it, N)], lhsT=(qw_t[:]), rhs=(s["h"][:, ts(it, N)]),
                        start=True, stop=True,
                    )
                y2 = ypool.tile([A, 2 * N], F32, tag="y")
                nc.vector.tensor_scalar_add(y2[:], yp2[:], qb_t[:])
                nc.sync.dma_start(
                    out=yt_d[2 * g : 2 * g + 2].rearrange("i a n -> a i n"),
                    in_=y2[:],
                )

            def stage_CD(g, p):
                stage_C(g, p)
                stage_D(g, p)

            def emit(g, sidx):
                if sidx == 0:
                    stage_entry(g)
                elif sidx == 7:
                    stage_head(g)
                else:
                    p, sub = divmod(sidx - 1, 3)
                    [stage_A, stage_B, stage_CD][sub](g, p)

            # 8 stages per pair, pairs skewed by 4 slots: pair g's CD stage
            # (the serial softmax-normalize chain) always co-slots with pair
            # g+1's B stage (scores matmuls), so the PE never starves.
            NPAIR = IPC // 2
            NSTAGE, SKEW = 8, 4
            for t in range(NSTAGE + SKEW * (NPAIR - 1)):
                for g in range(NPAIR):  # older (further-along) pair first
                    sidx = t - SKEW * g
                    if 0 <= sidx < NSTAGE:
                        emit(g, sidx)

    _spill_excess_waits(nc)
    return nc


_prog_cache = None


def _get_program():
    global _prog_cache
    if _prog_cache is None:
        _prog_cache = build_program()
    return _prog_cache


def _make_in_maps(x, mask, enc_w, enc_b, wv, bv, wk, bk, wq, bq, wo, bo, qw, qb):
    import ml_dtypes
    bf = lambda a: np.ascontiguousarray(np.asarray(a, dtype=np.float32).astype(ml_dtypes.bfloat16))
    f = lambda a: np.ascontiguousarray(np.asarray(a, dtype=np.float32))
    x, mask = f(x), f(mask)
    shared = {
        "enc_w": bf(enc_w),
        "enc_b": f(enc_b).reshape(H, 1),
        "wq": bf(wq),
        "wk": bf(wk),
        "wv": bf(wv),
        "wo": bf(wo),
        "bq": f(bq).reshape(P, H, 1),
        "bk": f(bk).reshape(P, H, 1),
        "bv4": np.ascontiguousarray(np.tile(bf(bv), (1, NCH)).reshape(P, 1, NCH * H)),
        "bo": f(bo).reshape(P, H, 1),
        "qw": bf(qw),
        "ones": np.ones((128, 128), dtype=ml_dtypes.bfloat16),
        "qb": f(qb).reshape(A, 1),
    }
    in_maps = []
    for c in range(N_CORES):
        sl = slice(c * IPC, (c + 1) * IPC)
        in_maps.append({
            "xt": np.ascontiguousarray(x[sl].transpose(0, 2, 1).astype(ml_dtypes.bfloat16)),
            "maskt": np.ascontiguousarray(mask[sl].transpose(0, 2, 1).astype(ml_dtypes.bfloat16)),
            **shared,
        })
    return in_maps


def run(trace=False, **inputs):
    nc = _get_program()
    in_maps = _make_in_maps(**inputs)
    res = run_bass_kernel_spmd(nc, in_maps, list(range(N_CORES)), trace=trace)
    y = np.concatenate(
        [r["yt"].transpose(0, 2, 1) for r in res.results], axis=0
    ).astype(np.float32)
    return y, res


def kernel(**inputs):
    y, _ = run(trace=False, **inputs)
    return y


# revision 28
# speedup vs baseline: 1.4333x; 1.0684x over previous
"""Trainium2 Bass kernel for the DGN message-passing network.

Computation (per batch item b):
    h = relu(x @ enc_w + enc_b)                      [N, H]
    for p in 0..P-1:
        v = relu(h @ wv[p] + bv[p]); q = relu(h @ wq[p] + bq[p]); k = relu(h @ wk[p] + bk[p])
        att = softmax(q @ k.T  masked by mask, axis=-1)
        h = relu((att @ v) @ wo[p] + bo[p])
    y = h @ qw + qb                                  [N, A]

Sharding: data-parallel over the batch dim across 8 NeuronCores (16 items
per core), weights replicated, no cross-core communication.

On-chip layout: everything is kept transposed ([H, N] with H on partitions)
so no transposes are ever required:
  * hT/qT/kT = [H=128, N=512];   projections:  xT = wq.T @ hT  (lhsT = wq)
  * scoresT[m, n] = q[n]·k[m] computed directly as kT-chunk.T @ qT
  * softmax over m (= partition axis of scoresT) is done as
    exp(s)*mask -> rowsum via an all-ones [128,128] matmul (which lands the
    row-sum broadcast across all partitions) -> multiply by reciprocal.
    No max-subtraction: scores of this model are O(8), exp is safe, and
    softmax is shift-invariant so the result matches the reference.
  * v is needed m-on-partitions for the att@v contraction, so it is built
    natively as 4 row chunks packed in one [128, 4*H] PSUM tile; the bias
    (which varies along the free axis there) is preloaded with a single
    K=1 ones x bv4 matmul, then the 4 h-chunk matmuls accumulate on top.

Engine budget per pass-unit (16 items x 2 passes), targeting ~3.7us/unit
on every engine so the PE never starves (HAM stays warm at 2.4 GHz):
  PE : q,k MMs + v preload/4MM + 4 score MMs + 4 rowsum + 4 attv + out MM
  ACT: exp x2 (wide [128,1024]) + ln + exp(-ln) + q-relu (+ enc relu)
  DVE: k-relu, v-relu, h2-relu, otn mult, 2 mask mults (+ y bias-add)
  GPS: 2 mask mults
"""

import numpy as np

import concourse.bass as bass
import concourse.mybir as mybir
import concourse.tile as tile
from concourse.bass import ts
from concourse.bass_utils import run_bass_kernel_spmd

F32 = mybir.dt.float32
BF16 = mybir.dt.bfloat16
AF = mybir.ActivationFunctionType
OP = mybir.AluOpType

N_CORES = 8
B, N, DIN, H, P, A = 128, 512, 64, 128, 2, 16
IPC = B // N_CORES  # batch items per core
NCH = N // 128      # 128-row chunks of the agent dim


def _spill_excess_waits(nc):
    """Walrus codegen has limited sync-wait slots per instruction: a
    self-loading fp32/fp32r Matmult takes only 1 (waits land on its fused
    LDWEIGHTS micro-op) and sequencer ctrl ops (Drain/NoOp) take 4. Spill
    excess waits onto NoOps inserted just before the instruction on the same
    engine - the engine blocks at the NoOp, so ordering semantics are kept.
    """
    counter = [0]

    def make_nop(engine, waits):
        counter[0] += 1
        nop = mybir.InstNoOp(name=f"I-waitspill-{counter[0]}")
        nop.engine = engine
        nop.sync_info = mybir.SyncInfo(on_wait=list(waits), on_update=[])
        return nop

    def sem_clear_insts(inst):
        """This walrus build rejects EVENT_SEMAPHORE_RANGE_CLEAR ("ISA wrong
        length"); expand Tile's tail range-clear into per-sem writes."""
        first = inst.ant_dict["range_first"]
        last = inst.ant_dict["range_last"]
        res = []
        for s in range(first, last + 1):
            counter[0] += 1
            ev = mybir.InstEventSemaphore(name=f"I-semclear-{counter[0]}")
            ev.engine = inst.engine
            ev.sync_info = mybir.SyncInfo(
                on_wait=list(inst.sync_info.on_wait) if (s == first and inst.sync_info) else [],
                on_update=[mybir.SyncUpdate(
                    sync_type="semaphore", id=s,
                    update_mode="sem-wr-imm", update_value=0,
                )],
            )
            res.append(ev)
        return res

    for fn in nc.m.functions:
        for blk in fn.blocks:
            out = []
            for inst in blk.instructions:
                if (type(inst).__name__ == "InstISA"
                        and inst.ant_dict.get("header", {}).get("opcode") == 176):
                    out.extend(sem_clear_insts(inst))
                    continue
                si = inst.sync_info
                waits = list(si.on_wait) if si is not None else []
                limit = 1
                if len(waits) > limit:
                    keep = waits[-limit:] if limit else []
                    spill = waits[: len(waits) - limit]
                    for w in spill:
                        out.append(make_nop(inst.engine, [w]))
                    inst.sync_info.on_wait = keep
                out.append(inst)
            blk.instructions = out


def build_program():
    nc = bass.Bass("TRN2", target_bir_lowering=False, debug=False)

    xt_d = nc.dram_tensor("xt", [IPC, DIN, N], BF16, kind="ExternalInput").ap()
    mt_d = nc.dram_tensor("maskt", [IPC, N, N], BF16, kind="ExternalInput").ap()
    encw_d = nc.dram_tensor("enc_w", [DIN, H], BF16, kind="ExternalInput").ap()
    encb_d = nc.dram_tensor("enc_b", [H, 1], F32, kind="ExternalInput").ap()
    wq_d = nc.dram_tensor("wq", [P, H, H], BF16, kind="ExternalInput").ap()
    wk_d = nc.dram_tensor("wk", [P, H, H], BF16, kind="ExternalInput").ap()
    wv_d = nc.dram_tensor("wv", [P, H, H], BF16, kind="ExternalInput").ap()
    wo_d = nc.dram_tensor("wo", [P, H, H], BF16, kind="ExternalInput").ap()
    bq_d = nc.dram_tensor("bq", [P, H, 1], F32, kind="ExternalInput").ap()
    bk_d = nc.dram_tensor("bk", [P, H, 1], F32, kind="ExternalInput").ap()
    bv4_d = nc.dram_tensor("bv4", [P, 1, NCH * H], BF16, kind="ExternalInput").ap()
    bo_d = nc.dram_tensor("bo", [P, H, 1], F32, kind="ExternalInput").ap()
    qw_d = nc.dram_tensor("qw", [H, A], BF16, kind="ExternalInput").ap()
    ones_d = nc.dram_tensor("ones", [128, 128], BF16, kind="ExternalInput").ap()
    ident_d = nc.dram_tensor("ident", [128, 128], BF16, kind="ExternalInput").ap()
    qb_d = nc.dram_tensor("qb", [A, 1], F32, kind="ExternalInput").ap()
    yt_d = nc.dram_tensor("yt", [IPC, A, N], F32, kind="ExternalOutput").ap()

    from contextlib import ExitStack

    with tile.TileContext(nc) as tc:
        with ExitStack() as stack:
            ep = lambda p: stack.enter_context(p)
            wpool = ep(tc.tile_pool(name="weights", bufs=1))
            xpool = ep(tc.tile_pool(name="xin", bufs=2))
            mpool = ep(tc.tile_pool(name="maskin", bufs=4))
            hpool = ep(tc.tile_pool(name="hbuf", bufs=2))
            qpool = ep(tc.tile_pool(name="qbuf", bufs=3))
            kpool = ep(tc.tile_pool(name="kbuf", bufs=3))
            vpool = ep(tc.tile_pool(name="vbuf", bufs=3))
            ppool = ep(tc.tile_pool(name="pbuf", bufs=3))
            rpool = ep(tc.tile_pool(name="rbuf", bufs=2))
            opool = ep(tc.tile_pool(name="obuf", bufs=3))
            ypool = ep(tc.tile_pool(name="ybuf", bufs=2))
            # PSUM: 8 banks exactly, four 2-bank [128,1024] tags x 1 buf.
            # Slot reuse is gated on an early consumer of the previous
            # lockstep stage, so the pair pipeline never blocks on banks:
            #   qkh: enc2(g) -> qk(i0) -> qk(i1) -> h2p2 -> ... -> enc2(g+1)
            #   sc : v(i0) -> v(i1) -> sc(i0,j0..j1) -> sc(i1,j0..j1) -> yp2
            #   rs2/ot2: pair-wide rowsum / att@v accumulators
            qkpsum = ep(tc.tile_pool(name="qkpsum", bufs=1, space="PSUM"))
            spsum = ep(tc.tile_pool(name="spsum", bufs=1, space="PSUM"))
            rpsum = ep(tc.tile_pool(name="rpsum", bufs=1, space="PSUM"))
            opsum = ep(tc.tile_pool(name="opsum", bufs=1, space="PSUM"))
            # ---- resident weights ----
            encw_t = wpool.tile([DIN, H], BF16, tag="encw")
            nc.sync.dma_start(out=encw_t[:], in_=encw_d[:])
            encb_t = wpool.tile([H, 1], F32, tag="encb")
            nc.sync.dma_start(out=encb_t[:], in_=encb_d[:])
            qw_t = wpool.tile([H, A], BF16, tag="qw")
            nc.sync.dma_start(out=qw_t[:], in_=qw_d[:])
            qb_t = wpool.tile([A, 1], F32, tag="qb")
            nc.sync.dma_start(out=qb_t[:], in_=qb_d[:])
            ones_t = wpool.tile([128, 128], BF16, tag="ones")
            nc.sync.dma_start(out=ones_t[:], in_=ones_d[:])
            ident_t = wpool.tile([128, 128], BF16, tag="ident")
            nc.sync.dma_start(out=ident_t[:], in_=ident_d[:])

            wq_t, wk_t, wv_t, wo_t, bq_t, bk_t, bv4_t, bo_t = [], [], [], [], [], [], [], []
            for p in range(P):
                for lst, dram, shape, tag, dt in (
                    (wq_t, wq_d, [H, H], "wq", BF16),
                    (wk_t, wk_d, [H, H], "wk", BF16),
                    (wv_t, wv_d, [H, H], "wv", BF16),
                    (wo_t, wo_d, [H, H], "wo", BF16),
                    (bq_t, bq_d, [H, 1], "bq", F32),
                    (bk_t, bk_d, [H, 1], "bk", F32),
                    (bv4_t, bv4_d, [1, NCH * H], "bv4", BF16),
                    (bo_t, bo_d, [H, 1], "bo", F32),
                ):
                    t = wpool.tile(shape, dt, tag=f"{tag}{p}")
                    nc.sync.dma_start(out=t[:], in_=dram[p])
                    lst.append(t)

            # ---- two-stream skewed item-PAIR pipeline ----
            # Items march in lockstep pairs, and TWO pairs are kept in
            # flight, skewed by SKEW=5 stage-slots (pair g at pass-1 while
            # pair g+1 runs pass-0). Each pair's serial normalize chain
            # (ln -> exp -> otn -> h2) then overlaps the other pair's
            # matmul-dense stages, so the PE never starves and the HAM
            # clock gate stays warm. PSUM tags rotate just-in-time under
            # this emission order (verified per-tag; all bufs=1).
            st = [dict() for _ in range(IPC // 2)]

            def stage_entry(g):
                s = st[g]
                xp_t = xpool.tile([DIN, 2 * N], BF16, tag="xt")
                nc.sync.dma_start(
                    out=xp_t[:],
                    in_=xt_d[2 * g : 2 * g + 2].rearrange("i d n -> d i n"),
                )
                s["mt"] = []
                for it in range(2):
                    t = mpool.tile([128, NCH * N], BF16, tag="mt")
                    nc.sync.dma_start(
                        out=t[:],
                        in_=mt_d[2 * g + it].rearrange("(c p) n -> p c n", c=NCH),
                    )
                    s["mt"].append(t)
                # encoder for the pair: hpair = relu(enc_w.T @ [x0|x1] + b)
                ep = qkpsum.tile([H, 2 * N], F32, tag="qkh")
                for it in range(2):
                    nc.tensor.matmul(
                        ep[:, ts(it, N)], lhsT=(encw_t[:]), rhs=(xp_t[:, ts(it, N)]),
                        start=True, stop=True,
                    )
                hpair = hpool.tile([H, 2 * N], BF16, tag="h")
                nc.vector.tensor_scalar(
                    out=hpair[:], in0=ep[:], scalar1=encb_t[:], scalar2=0.0,
                    op0=OP.add, op1=OP.max,
                )
                s["h"] = hpair

            def stage_A(g, p):
                s = st[g]
                hpair = s["h"]
                s["q"], s["k"], s["v"] = [], [], []
                for it in range(2):
                    hT = hpair[:, ts(it, N)]
                    qkp = qkpsum.tile([H, 2 * N], F32, tag="qkh")
                    nc.tensor.matmul(qkp[:, 0:N], lhsT=(wq_t[p][:]), rhs=(hT), start=True, stop=True)
                    nc.tensor.matmul(qkp[:, N:], lhsT=(wk_t[p][:]), rhs=(hT), start=True, stop=True)
                    qt = qpool.tile([H, N], BF16, tag="q")
                    nc.vector.tensor_scalar(
                        out=qt[:], in0=qkp[:, 0:N], scalar1=bq_t[p][:], scalar2=0.0,
                        op0=OP.add, op1=OP.max,
                    )
                    s["q"].append(qt)
                    kt = kpool.tile([H, N], BF16, tag="k")
                    nc.vector.tensor_scalar(
                        out=kt[:], in0=qkp[:, N:], scalar1=bk_t[p][:], scalar2=0.0,
                        op0=OP.add, op1=OP.max,
                    )
                    s["k"].append(kt)
                    # v in natural [m, h] layout: 4 row chunks in one
                    # [128, 4H] PSUM tile; bias varies along the FREE
                    # axis, so preload ones(x)bv4 via a K=1 matmul.
                    vp = spsum.tile([128, NCH * H], F32, tag="sc")
                    nc.tensor.matmul(
                        vp[:], lhsT=(ones_t[0:1, :]), rhs=(bv4_t[p][:]),
                        start=True, stop=False,
                    )
                    for c in range(NCH):
                        nc.tensor.matmul(
                            vp[:, ts(c, H)], lhsT=(hT[:, ts(c, 128)]), rhs=(wv_t[p][:]),
                            start=False, stop=(c == NCH - 1),
                        )
                    vt = vpool.tile([128, NCH * H], BF16, tag="v")
                    nc.vector.tensor_scalar_max(vt[:], vp[:], 0.0)
                    s["v"].append(vt)

            def stage_B(g, p):
                # The mask is folded into the scores PSUM before exp: the
                # host ships maskneg = -30000*(1-mask), and an identity
                # matmul accumulates it onto kT.T @ qT (out += I.T @ maskneg
                # = maskneg elementwise). exp then yields exactly-masked
                # attention weights with NO elementwise mask op - the chain
                # from exp to rowsum/attv is pure ACT -> PE.
                s = st[g]
                s["p"] = []
                for it in range(2):
                    pt = ppool.tile([128, NCH * N], BF16, tag="p")
                    for j in range(NCH // 2):
                        scp = spsum.tile([128, 2 * N], F32, tag="sc")
                        for cc in range(2):
                            c = 2 * j + cc
                            nc.tensor.matmul(
                                scp[:, ts(cc, N)], lhsT=(s["k"][it][:, ts(c, 128)]),
                                rhs=(s["q"][it][:]), start=True, stop=False,
                            )
                            nc.tensor.matmul(
                                scp[:, ts(cc, N)], lhsT=(ident_t[:]),
                                rhs=(s["mt"][it][:, ts(c, N)]), start=False, stop=True,
                            )
                        nc.scalar.activation(pt[:, ts(j, 2 * N)], scp[:], AF.Exp)
                    s["p"].append(pt)

            def stage_C(g, p):
                s = st[g]
                rs2 = rpsum.tile([128, 2 * N], F32, tag="rs2")
                for it in range(2):
                    for c in range(NCH):
                        nc.tensor.matmul(
                            rs2[:, it * N : (it + 1) * N], lhsT=(ones_t[:]),
                            rhs=(s["p"][it][:, ts(c, N)]),
                            start=(c == 0), stop=(c == NCH - 1),
                        )
                ot2 = opsum.tile([H, 2 * N], F32, tag="ot2")
                for it in range(2):
                    for c in range(NCH):
                        nc.tensor.matmul(
                            ot2[:, it * N : (it + 1) * N], lhsT=(s["v"][it][:, ts(c, H)]),
                            rhs=(s["p"][it][:, ts(c, N)]),
                            start=(c == 0), stop=(c == NCH - 1),
                        )
                s["rs2"], s["ot2"] = rs2, ot2

            def stage_D(g, p):
                s = st[g]
                # 1/rowsum = exp(-ln(rowsum)): Ln/Exp share the loaded ACT
                # table set (Reciprocal would thrash 2.7us table loads).
                # Fully per-item chains: item i0's ln starts as soon as its
                # own 4 rowsum matmuls retire, and its h2-relu unblocks the
                # next A-stage while i1 is still normalizing.
                for it in range(2):
                    lnr = rpool.tile([H, N], F32, tag="lnr")
                    nc.scalar.activation(lnr[:], s["rs2"][:, ts(it, N)], AF.Ln)
                    recip = rpool.tile([H, N], F32, tag="recip")
                    nc.scalar.activation(recip[:], lnr[:], AF.Exp, scale=-1.0)
                    otn = opool.tile([H, N], BF16, tag="otn")
                    nc.vector.tensor_tensor(
                        out=otn[:], in0=s["ot2"][:, ts(it, N)],
                        in1=recip[:], op=OP.mult,
                    )
                    h2p = qkpsum.tile([H, N], F32, tag="qkh")
                    nc.tensor.matmul(h2p[:], lhsT=(wo_t[p][:]), rhs=(otn[:]), start=True, stop=True)
                    # overwrite hpair half in place (pass-p readers done)
                    nc.vector.tensor_scalar(
                        out=s["h"][:, ts(it, N)], in0=h2p[:], scalar1=bo_t[p][:],
                        scalar2=0.0, op0=OP.add, op1=OP.max,
                    )

            def stage_head(g):
                s = st[g]
                yp2 = spsum.tile([A, 2 * N], F32, tag="sc")
                for it in range(2):
                    nc.tensor.matmul(
                        yp2[:, ts(it, N)], lhsT=(qw_t[:]), rhs=(s["h"][:, ts(it, N)]),
                        start=True, stop=True,
                    )
                y2 = ypool.tile([A, 2 * N], F32, tag="y")
                nc.vector.tensor_scalar_add(y2[:], yp2[:], qb_t[:])
                nc.sync.dma_start(
                    out=yt_d[2 * g : 2 * g + 2].rearrange("i a n -> a i n"),
                    in_=y2[:],
                )

            def stage_CD(g, p):
                stage_C(g, p)
                stage_D(g, p)

            def emit(g, sidx):
                if sidx == 0:
                    stage_entry(g)
                elif sidx == 7:
                    stage_head(g)
                else:
                    p, sub = divmod(sidx - 1, 3)
                    [stage_A, stage_B, stage_CD][sub](g, p)

            # 8 stages per pair, pairs skewed by 4 slots: pair g's CD stage
            # (the serial softmax-normalize chain) always co-slots with pair
            # g+1's B stage (scores matmuls), so the PE never starves.
            NPAIR = IPC // 2
            NSTAGE, SKEW = 8, 4
            for t in range(NSTAGE + SKEW * (NPAIR - 1)):
                for g in range(NPAIR):  # older (further-along) pair first
                    sidx = t - SKEW * g
                    if 0 <= sidx < NSTAGE:
                        emit(g, sidx)

    _spill_excess_waits(nc)
    return nc


_prog_cache = None


def _get_program():
    global _prog_cache
    if _prog_cache is None:
        _prog_cache = build_program()
    return _prog_cache


def _make_in_maps(x, mask, enc_w, enc_b, wv, bv, wk, bk, wq, bq, wo, bo, qw, qb):
    import ml_dtypes
    bf = lambda a: np.ascontiguousarray(np.asarray(a, dtype=np.float32).astype(ml_dtypes.bfloat16))
    f = lambda a: np.ascontiguousarray(np.asarray(a, dtype=np.float32))
    x, mask = f(x), f(mask)
    shared = {
        "enc_w": bf(enc_w),
        "enc_b": f(enc_b).reshape(H, 1),
        "wq": bf(wq),
        "wk": bf(wk),
        "wv": bf(wv),
        "wo": bf(wo),
        "bq": f(bq).reshape(P, H, 1),
        "bk": f(bk).reshape(P, H, 1),
        "bv4": np.ascontiguousarray(np.tile(bf(bv), (1, NCH)).reshape(P, 1, NCH * H)),
        "bo": f(bo).reshape(P, H, 1),
        "qw": bf(qw),
        "ones": np.ones((128, 128), dtype=ml_dtypes.bfloat16),
        "ident": np.eye(128, dtype=ml_dtypes.bfloat16),
        "qb": f(qb).reshape(A, 1),
    }
    # Additive mask: -30000 where mask==0, 0 where mask==1. Injected into
    # the scores PSUM by an identity matmul; exp(-30000) == 0 exactly.
    maskneg = (-30000.0 * (1.0 - mask)).astype(ml_dtypes.bfloat16)
    in_maps = []
    for c in range(N_CORES):
        sl = slice(c * IPC, (c + 1) * IPC)
        in_maps.append({
            "xt": np.ascontiguousarray(x[sl].transpose(0, 2, 1).astype(ml_dtypes.bfloat16)),
            "maskt": np.ascontiguousarray(maskneg[sl].transpose(0, 2, 1)),
            **shared,
        })
    return in_maps


def run(trace=False, **inputs):
    nc = _get_program()
    in_maps = _make_in_maps(**inputs)
    res = run_bass_kernel_spmd(nc, in_maps, list(range(N_CORES)), trace=trace)
    y = np.concatenate(
        [r["yt"].transpose(0, 2, 1) for r in res.results], axis=0
    ).astype(np.float32)
    return y, res


def kernel(**inputs):
    y, _ = run(trace=False, **inputs)
    return y


# revision 30
# speedup vs baseline: 1.6816x; 1.1733x over previous
"""Trainium2 Bass kernel for the DGN message-passing network.

Computation (per batch item b):
    h = relu(x @ enc_w + enc_b)                      [N, H]
    for p in 0..P-1:
        v = relu(h @ wv[p] + bv[p]); q = relu(h @ wq[p] + bq[p]); k = relu(h @ wk[p] + bk[p])
        att = softmax(q @ k.T  masked by mask, axis=-1)
        h = relu((att @ v) @ wo[p] + bo[p])
    y = h @ qw + qb                                  [N, A]

Sharding: data-parallel over the batch dim across 8 NeuronCores (16 items
per core), weights replicated, no cross-core communication.

On-chip layout: everything is kept transposed ([H, N] with H on partitions)
so no transposes are ever required:
  * hT/qT/kT = [H=128, N=512];   projections:  xT = wq.T @ hT  (lhsT = wq)
  * scoresT[m, n] = q[n]·k[m] computed directly as kT-chunk.T @ qT
  * softmax over m (= partition axis of scoresT) is done as
    exp(s)*mask -> rowsum via an all-ones [128,128] matmul (which lands the
    row-sum broadcast across all partitions) -> multiply by reciprocal.
    No max-subtraction: scores of this model are O(8), exp is safe, and
    softmax is shift-invariant so the result matches the reference.
  * v is needed m-on-partitions for the att@v contraction, so it is built
    natively as 4 row chunks packed in one [128, 4*H] PSUM tile; the bias
    (which varies along the free axis there) is preloaded with a single
    K=1 ones x bv4 matmul, then the 4 h-chunk matmuls accumulate on top.

Engine budget per pass-unit (16 items x 2 passes), targeting ~3.7us/unit
on every engine so the PE never starves (HAM stays warm at 2.4 GHz):
  PE : q,k MMs + v preload/4MM + 4 score MMs + 4 rowsum + 4 attv + out MM
  ACT: exp x2 (wide [128,1024]) + ln + exp(-ln) + q-relu (+ enc relu)
  DVE: k-relu, v-relu, h2-relu, otn mult, 2 mask mults (+ y bias-add)
  GPS: 2 mask mults
"""

import numpy as np

import concourse.bass as bass
import concourse.mybir as mybir
import concourse.tile as tile
from concourse.bass import ts
from concourse.bass_utils import run_bass_kernel_spmd

F32 = mybir.dt.float32
BF16 = mybir.dt.bfloat16
AF = mybir.ActivationFunctionType
OP = mybir.AluOpType

N_CORES = 8
B, N, DIN, H, P, A = 128, 512, 64, 128, 2, 16
IPC = B // N_CORES  # batch items per core
NCH = N // 128      # 128-row chunks of the agent dim


def _spill_excess_waits(nc):
    """Walrus codegen has limited sync-wait slots per instruction: a
    self-loading fp32/fp32r Matmult takes only 1 (waits land on its fused
    LDWEIGHTS micro-op) and sequencer ctrl ops (Drain/NoOp) take 4. Spill
    excess waits onto NoOps inserted just before the instruction on the same
    engine - the engine blocks at the NoOp, so ordering semantics are kept.
    """
    counter = [0]

    def make_nop(engine, waits):
        counter[0] += 1
        nop = mybir.InstNoOp(name=f"I-waitspill-{counter[0]}")
        nop.engine = engine
        nop.sync_info = mybir.SyncInfo(on_wait=list(waits), on_update=[])
        return nop

    def sem_clear_insts(inst):
        """This walrus build rejects EVENT_SEMAPHORE_RANGE_CLEAR ("ISA wrong
        length"); expand Tile's tail range-clear into per-sem writes."""
        first = inst.ant_dict["range_first"]
        last = inst.ant_dict["range_last"]
        res = []
        for s in range(first, last + 1):
            counter[0] += 1
            ev = mybir.InstEventSemaphore(name=f"I-semclear-{counter[0]}")
            ev.engine = inst.engine
            ev.sync_info = mybir.SyncInfo(
                on_wait=list(inst.sync_info.on_wait) if (s == first and inst.sync_info) else [],
                on_update=[mybir.SyncUpdate(
                    sync_type="semaphore", id=s,
                    update_mode="sem-wr-imm", update_value=0,
                )],
            )
            res.append(ev)
        return res

    for fn in nc.m.functions:
        for blk in fn.blocks:
            out = []
            for inst in blk.instructions:
                if (type(inst).__name__ == "InstISA"
                        and inst.ant_dict.get("header", {}).get("opcode") == 176):
                    out.extend(sem_clear_insts(inst))
                    continue
                si = inst.sync_info
                waits = list(si.on_wait) if si is not None else []
                limit = 1
                if len(waits) > limit:
                    keep = waits[-limit:] if limit else []
                    spill = waits[: len(waits) - limit]
                    for w in spill:
                        out.append(make_nop(inst.engine, [w]))
                    inst.sync_info.on_wait = keep
                out.append(inst)
            blk.instructions = out


def build_program():
    nc = bass.Bass("TRN2", target_bir_lowering=False, debug=False)

    xt_d = nc.dram_tensor("xt", [IPC, DIN, N], BF16, kind="ExternalInput").ap()
    mt_d = nc.dram_tensor("maskt", [IPC, N, N], BF16, kind="ExternalInput").ap()
    encw_d = nc.dram_tensor("enc_w", [DIN, H], BF16, kind="ExternalInput").ap()
    encb_d = nc.dram_tensor("enc_b", [H, 1], F32, kind="ExternalInput").ap()
    wq_d = nc.dram_tensor("wq", [P, H, H], BF16, kind="ExternalInput").ap()
    wk_d = nc.dram_tensor("wk", [P, H, H], BF16, kind="ExternalInput").ap()
    wv_d = nc.dram_tensor("wv", [P, H, H], BF16, kind="ExternalInput").ap()
    wo_d = nc.dram_tensor("wo", [P, H, H], BF16, kind="ExternalInput").ap()
    bq_d = nc.dram_tensor("bq", [P, H, 1], F32, kind="ExternalInput").ap()
    bk_d = nc.dram_tensor("bk", [P, H, 1], F32, kind="ExternalInput").ap()
    bv4_d = nc.dram_tensor("bv4", [P, 1, NCH * H], BF16, kind="ExternalInput").ap()
    bo_d = nc.dram_tensor("bo", [P, H, 1], F32, kind="ExternalInput").ap()
    qw_d = nc.dram_tensor("qw", [H, A], BF16, kind="ExternalInput").ap()
    ones_d = nc.dram_tensor("ones", [128, 128], BF16, kind="ExternalInput").ap()
    ident_d = nc.dram_tensor("ident", [128, 128], BF16, kind="ExternalInput").ap()
    qb_d = nc.dram_tensor("qb", [A, 1], F32, kind="ExternalInput").ap()
    yt_d = nc.dram_tensor("yt", [IPC, A, N], F32, kind="ExternalOutput").ap()

    from contextlib import ExitStack

    with tile.TileContext(nc) as tc:
        with ExitStack() as stack:
            ep = lambda p: stack.enter_context(p)
            wpool = ep(tc.tile_pool(name="weights", bufs=1))
            xpool = ep(tc.tile_pool(name="xin", bufs=2))
            mpool = ep(tc.tile_pool(name="maskin", bufs=4))
            hpool = ep(tc.tile_pool(name="hbuf", bufs=2))
            qpool = ep(tc.tile_pool(name="qbuf", bufs=3))
            kpool = ep(tc.tile_pool(name="kbuf", bufs=3))
            vpool = ep(tc.tile_pool(name="vbuf", bufs=3))
            ppool = ep(tc.tile_pool(name="pbuf", bufs=3))
            rpool = ep(tc.tile_pool(name="rbuf", bufs=2))
            opool = ep(tc.tile_pool(name="obuf", bufs=3))
            ypool = ep(tc.tile_pool(name="ybuf", bufs=2))
            # PSUM: 8 banks exactly, four 2-bank [128,1024] tags x 1 buf.
            # Slot reuse is gated on an early consumer of the previous
            # lockstep stage, so the pair pipeline never blocks on banks:
            #   qkh: enc2(g) -> qk(i0) -> qk(i1) -> h2p2 -> ... -> enc2(g+1)
            #   sc : v(i0) -> v(i1) -> sc(i0,j0..j1) -> sc(i1,j0..j1) -> yp2
            #   rs2/ot2: pair-wide rowsum / att@v accumulators
            qkpsum = ep(tc.tile_pool(name="qkpsum", bufs=1, space="PSUM"))
            spsum = ep(tc.tile_pool(name="spsum", bufs=1, space="PSUM"))
            rpsum = ep(tc.tile_pool(name="rpsum", bufs=1, space="PSUM"))
            opsum = ep(tc.tile_pool(name="opsum", bufs=1, space="PSUM"))
            # ---- resident weights ----
            encw_t = wpool.tile([DIN, H], BF16, tag="encw")
            nc.sync.dma_start(out=encw_t[:], in_=encw_d[:])
            encb_t = wpool.tile([H, 1], F32, tag="encb")
            nc.sync.dma_start(out=encb_t[:], in_=encb_d[:])
            qw_t = wpool.tile([H, A], BF16, tag="qw")
            nc.sync.dma_start(out=qw_t[:], in_=qw_d[:])
            qb_t = wpool.tile([A, 1], F32, tag="qb")
            nc.sync.dma_start(out=qb_t[:], in_=qb_d[:])
            ones_t = wpool.tile([128, 128], BF16, tag="ones")
            nc.sync.dma_start(out=ones_t[:], in_=ones_d[:])
            ident_t = wpool.tile([128, 128], BF16, tag="ident")
            nc.sync.dma_start(out=ident_t[:], in_=ident_d[:])

            wq_t, wk_t, wv_t, wo_t, bq_t, bk_t, bv4_t, bo_t = [], [], [], [], [], [], [], []
            for p in range(P):
                for lst, dram, shape, tag, dt in (
                    (wq_t, wq_d, [H, H], "wq", BF16),
                    (wk_t, wk_d, [H, H], "wk", BF16),
                    (wv_t, wv_d, [H, H], "wv", BF16),
                    (wo_t, wo_d, [H, H], "wo", BF16),
                    (bq_t, bq_d, [H, 1], "bq", F32),
                    (bk_t, bk_d, [H, 1], "bk", F32),
                    (bv4_t, bv4_d, [1, NCH * H], "bv4", BF16),
                    (bo_t, bo_d, [H, 1], "bo", F32),
                ):
                    t = wpool.tile(shape, dt, tag=f"{tag}{p}")
                    nc.sync.dma_start(out=t[:], in_=dram[p])
                    lst.append(t)

            # ---- two-stream skewed item-PAIR pipeline ----
            # Items march in lockstep pairs, and TWO pairs are kept in
            # flight, skewed by SKEW=5 stage-slots (pair g at pass-1 while
            # pair g+1 runs pass-0). Each pair's serial normalize chain
            # (ln -> exp -> otn -> h2) then overlaps the other pair's
            # matmul-dense stages, so the PE never starves and the HAM
            # clock gate stays warm. PSUM tags rotate just-in-time under
            # this emission order (verified per-tag; all bufs=1).
            st = [dict() for _ in range(IPC // 2)]

            def stage_entry(g):
                s = st[g]
                xp_t = xpool.tile([DIN, 2 * N], BF16, tag="xt")
                nc.sync.dma_start(
                    out=xp_t[:],
                    in_=xt_d[2 * g : 2 * g + 2].rearrange("i d n -> d i n"),
                )
                s["mt"] = []
                for it in range(2):
                    t = mpool.tile([128, NCH * N], BF16, tag="mt")
                    nc.sync.dma_start(
                        out=t[:],
                        in_=mt_d[2 * g + it].rearrange("(c p) n -> p c n", c=NCH),
                    )
                    s["mt"].append(t)
                # encoder for the pair: hpair = relu(enc_w.T @ [x0|x1] + b)
                ep = qkpsum.tile([H, 2 * N], F32, tag="qkh")
                for it in range(2):
                    nc.tensor.matmul(
                        ep[:, ts(it, N)], lhsT=(encw_t[:]), rhs=(xp_t[:, ts(it, N)]),
                        start=True, stop=True,
                    )
                hpair = hpool.tile([H, 2 * N], BF16, tag="h")
                nc.vector.tensor_scalar(
                    out=hpair[:], in0=ep[:], scalar1=encb_t[:], scalar2=0.0,
                    op0=OP.add, op1=OP.max,
                )
                s["h"] = hpair

            def stage_A(g, p):
                s = st[g]
                hpair = s["h"]
                s["q"], s["k"], s["v"] = [], [], []
                for it in range(2):
                    hT = hpair[:, ts(it, N)]
                    qkp = qkpsum.tile([H, 2 * N], F32, tag="qkh")
                    nc.tensor.matmul(qkp[:, 0:N], lhsT=(wq_t[p][:]), rhs=(hT), start=True, stop=True)
                    nc.tensor.matmul(qkp[:, N:], lhsT=(wk_t[p][:]), rhs=(hT), start=True, stop=True)
                    qt = qpool.tile([H, N], BF16, tag="q")
                    nc.vector.tensor_scalar(
                        out=qt[:], in0=qkp[:, 0:N], scalar1=bq_t[p][:], scalar2=0.0,
                        op0=OP.add, op1=OP.max,
                    )
                    s["q"].append(qt)
                    kt = kpool.tile([H, N], BF16, tag="k")
                    nc.vector.tensor_scalar(
                        out=kt[:], in0=qkp[:, N:], scalar1=bk_t[p][:], scalar2=0.0,
                        op0=OP.add, op1=OP.max,
                    )
                    s["k"].append(kt)
                    # v in natural [m, h] layout: 4 row chunks in one
                    # [128, 4H] PSUM tile; bias varies along the FREE
                    # axis, so preload ones(x)bv4 via a K=1 matmul.
                    vp = spsum.tile([128, NCH * H], F32, tag="sc")
                    nc.tensor.matmul(
                        vp[:], lhsT=(ones_t[0:1, :]), rhs=(bv4_t[p][:]),
                        start=True, stop=False,
                    )
                    for c in range(NCH):
                        nc.tensor.matmul(
                            vp[:, ts(c, H)], lhsT=(hT[:, ts(c, 128)]), rhs=(wv_t[p][:]),
                            start=False, stop=(c == NCH - 1),
                        )
                    vt = vpool.tile([128, NCH * H], BF16, tag="v")
                    nc.vector.tensor_scalar_max(vt[:], vp[:], 0.0)
                    s["v"].append(vt)

            def stage_B(g, p):
                # The mask is folded into the scores PSUM before exp: the
                # host ships maskneg = -30000*(1-mask), and an identity
                # matmul accumulates it onto kT.T @ qT (out += I.T @ maskneg
                # = maskneg elementwise). exp then yields exactly-masked
                # attention weights with NO elementwise mask op - the chain
                # from exp to rowsum/attv is pure ACT -> PE.
                s = st[g]
                s["p"] = []
                for it in range(2):
                    pt = ppool.tile([128, NCH * N], BF16, tag="p")
                    for c in range(NCH):
                        scp = spsum.tile([128, N], F32, tag="sc")
                        nc.tensor.matmul(
                            scp[:], lhsT=(s["k"][it][:, ts(c, 128)]),
                            rhs=(s["q"][it][:]), start=True, stop=False,
                        )
                        nc.tensor.matmul(
                            scp[:], lhsT=(ident_t[:]),
                            rhs=(s["mt"][it][:, ts(c, N)]), start=False, stop=True,
                        )
                        nc.scalar.activation(pt[:, ts(c, N)], scp[:], AF.Exp)
                    s["p"].append(pt)

            def stage_C(g, p):
                s = st[g]
                rs2 = rpsum.tile([128, 2 * N], F32, tag="rs2")
                for it in range(2):
                    for c in range(NCH):
                        nc.tensor.matmul(
                            rs2[:, it * N : (it + 1) * N], lhsT=(ones_t[:]),
                            rhs=(s["p"][it][:, ts(c, N)]),
                            start=(c == 0), stop=(c == NCH - 1),
                        )
                ot2 = opsum.tile([H, 2 * N], F32, tag="ot2")
                for it in range(2):
                    for c in range(NCH):
                        nc.tensor.matmul(
                            ot2[:, it * N : (it + 1) * N], lhsT=(s["v"][it][:, ts(c, H)]),
                            rhs=(s["p"][it][:, ts(c, N)]),
                            start=(c == 0), stop=(c == NCH - 1),
                        )
                s["rs2"], s["ot2"] = rs2, ot2

            def stage_D(g, p):
                s = st[g]
                # 1/rowsum = exp(-ln(rowsum)): Ln/Exp share the loaded ACT
                # table set (Reciprocal would thrash 2.7us table loads).
                # Fully per-item chains: item i0's ln starts as soon as its
                # own 4 rowsum matmuls retire, and its h2-relu unblocks the
                # next A-stage while i1 is still normalizing.
                for it in range(2):
                    lnr = rpool.tile([H, N], F32, tag="lnr")
                    nc.scalar.activation(lnr[:], s["rs2"][:, ts(it, N)], AF.Ln)
                    recip = rpool.tile([H, N], F32, tag="recip")
                    nc.scalar.activation(recip[:], lnr[:], AF.Exp, scale=-1.0)
                    otn = opool.tile([H, N], BF16, tag="otn")
                    nc.vector.tensor_tensor(
                        out=otn[:], in0=s["ot2"][:, ts(it, N)],
                        in1=recip[:], op=OP.mult,
                    )
                    h2p = qkpsum.tile([H, N], F32, tag="qkh")
                    nc.tensor.matmul(h2p[:], lhsT=(wo_t[p][:]), rhs=(otn[:]), start=True, stop=True)
                    # overwrite hpair half in place (pass-p readers done)
                    nc.vector.tensor_scalar(
                        out=s["h"][:, ts(it, N)], in0=h2p[:], scalar1=bo_t[p][:],
                        scalar2=0.0, op0=OP.add, op1=OP.max,
                    )

            def stage_head(g):
                s = st[g]
                for it in range(2):
                    yp = spsum.tile([A, N], F32, tag="sc")
                    nc.tensor.matmul(
                        yp[:], lhsT=(qw_t[:]), rhs=(s["h"][:, ts(it, N)]),
                        start=True, stop=True,
                    )
                    y1 = ypool.tile([A, N], F32, tag="y")
                    nc.vector.tensor_scalar_add(y1[:], yp[:], qb_t[:])
                    nc.sync.dma_start(out=yt_d[2 * g + it], in_=y1[:])

            def stage_CD(g, p):
                stage_C(g, p)
                stage_D(g, p)

            def emit(g, sidx):
                if sidx == 0:
                    stage_entry(g)
                elif sidx == 7:
                    stage_head(g)
                else:
                    p, sub = divmod(sidx - 1, 3)
                    [stage_A, stage_B, stage_CD][sub](g, p)

            # 8 stages per pair, pairs skewed by 4 slots: pair g's CD stage
            # (the serial softmax-normalize chain) always co-slots with pair
            # g+1's B stage (scores matmuls), so the PE never starves.
            NPAIR = IPC // 2
            NSTAGE, SKEW = 8, 4
            for t in range(NSTAGE + SKEW * (NPAIR - 1)):
                for g in range(NPAIR):  # older (further-along) pair first
                    sidx = t - SKEW * g
                    if 0 <= sidx < NSTAGE:
                        emit(g, sidx)

    _spill_excess_waits(nc)
    return nc


_prog_cache = None


def _get_program():
    global _prog_cache
    if _prog_cache is None:
        _prog_cache = build_program()
    return _prog_cache


def _make_in_maps(x, mask, enc_w, enc_b, wv, bv, wk, bk, wq, bq, wo, bo, qw, qb):
    import ml_dtypes
    bf = lambda a: np.ascontiguousarray(np.asarray(a, dtype=np.float32).astype(ml_dtypes.bfloat16))
    f = lambda a: np.ascontiguousarray(np.asarray(a, dtype=np.float32))
    x, mask = f(x), f(mask)
    shared = {
        "enc_w": bf(enc_w),
        "enc_b": f(enc_b).reshape(H, 1),
        "wq": bf(wq),
        "wk": bf(wk),
        "wv": bf(wv),
        "wo": bf(wo),
        "bq": f(bq).reshape(P, H, 1),
        "bk": f(bk).reshape(P, H, 1),
        "bv4": np.ascontiguousarray(np.tile(bf(bv), (1, NCH)).reshape(P, 1, NCH * H)),
        "bo": f(bo).reshape(P, H, 1),
        "qw": bf(qw),
        "ones": np.ones((128, 128), dtype=ml_dtypes.bfloat16),
        "ident": np.eye(128, dtype=ml_dtypes.bfloat16),
        "qb": f(qb).reshape(A, 1),
    }
    # Additive mask: -30000 where mask==0, 0 where mask==1. Injected into
    # the scores PSUM by an identity matmul; exp(-30000) == 0 exactly.
    maskneg = (-30000.0 * (1.0 - mask)).astype(ml_dtypes.bfloat16)
    in_maps = []
    for c in range(N_CORES):
        sl = slice(c * IPC, (c + 1) * IPC)
        in_maps.append({
            "xt": np.ascontiguousarray(x[sl].transpose(0, 2, 1).astype(ml_dtypes.bfloat16)),
            "maskt": np.ascontiguousarray(maskneg[sl].transpose(0, 2, 1)),
            **shared,
        })
    return in_maps


def run(trace=False, **inputs):
    nc = _get_program()
    in_maps = _make_in_maps(**inputs)
    res = run_bass_kernel_spmd(nc, in_maps, list(range(N_CORES)), trace=trace)
    y = np.concatenate(
        [r["yt"].transpose(0, 2, 1) for r in res.results], axis=0
    ).astype(np.float32)
    return y, res


def kernel(**inputs):
    y, _ = run(trace=False, **inputs)
    return y
